# revision 22
# baseline (speedup 1.0000x reference)
"""Trainium2 Bass kernel for CustomRNN:
    h = tanh(x @ W1 + b1)                         [B,T,U]
    y_t = h_t + tanh(y_{t-1} @ W2 + b2)           (scan over T, y_{-1} = 0)

Strategy (8 NeuronCores, data-parallel over batch B=256 -> 32 rows/core):
  * All device-side layouts are transposed ([u/d on partitions, (step, batch)
    on free]) and the host does the transposes with numpy before/after the
    device call, so the kernel has zero on-chip transposes and every DMA has
    a 128-wide leading dim (the sim's DMA cost is bytes / leading-dim).
  * Split state: y_t = h_t + A_t with A_t = tanh(z_t),
    z_t = b2 + h_{t-1}@W2 + A_{t-1}@W2.
    The b2 term and h@W2 terms are batched into background GEMMs that
    deposit C_t = b2 + h_{t-1}@W2 for a whole 16-step group directly into a
    PSUM bank pair; the serial scan step is then only:
        4 small matmuls (A_{t-1}@W2, start=False accumulate onto C) -> tanh
    i.e. one PE->ACT->PE round trip per step; everything else (input
    projection GEMM + tanh, C GEMMs, y = h + A adds, DMA in/out) runs in
    engine-idle windows via a cost-budgeted background work queue.
  * When b1/b2 are all-zero (they are for this problem's inputs) the bias
    ones-matmuls are dropped; PSUM zero-regions are then initialized by the
    first C matmul (start=True marks the bank pending-zero, so later
    accumulating matmuls first-touch-overwrite).
  * f16 everywhere on-chip except PSUM accumulation (f32) and y (f32).
"""

import numpy as np

import concourse.bacc as bacc
import concourse.bass as bass
import concourse.mybir as mybir
import concourse.tile as tile
from concourse import bass_utils

F32 = mybir.dt.float32
F16 = mybir.dt.float16

B, T, D, U = 256, 512, 256, 256
NCORES = 8
BS = B // NCORES   # 32 batch rows per core
GR = 16            # scan steps per group
TB = GR * BS       # free columns per group (512), col = j*BS + b
P = 128

# background work-item budget costs (ns; ~2x actual so the drain paces at
# most one big PE op into each chain idle window)
C_HMM, C_ONES, C_J0, C_BIG, C_HACT = 427, 427, 30, 402, 398
PE_RATE, PE_CAP = 460, 1400
ACT_RATE, ACT_CAP = 420, 840


def build_rnn(T_steps=T, use_b1=True, use_b2=True):
    assert T_steps % GR == 0
    NG = T_steps // GR

    nc = bacc.Bacc("TRN2", debug=False)

    xT_d = nc.dram_tensor("xT", (NG, P, 2, TB), F16, kind="ExternalInput")
    W1_d = nc.dram_tensor("W1t", (P, 2, U), F16, kind="ExternalInput")
    b1_d = nc.dram_tensor("b1t", (P, 2), F32, kind="ExternalInput")
    W2_d = nc.dram_tensor("W2t", (P, 2, U), F16, kind="ExternalInput")
    b2_d = nc.dram_tensor("b2t", (1, U), F16, kind="ExternalInput")
    y_d = nc.dram_tensor("yT", (NG, P, 2, TB), F32, kind="ExternalOutput")
    ones_d = nc.inline_tensor(np.ones((1, TB), dtype=np.float16), "ones_row")

    with tile.TileContext(nc) as tc:
        with (
            tc.tile_pool(name="const", bufs=1) as cpool,
            tc.tile_pool(name="xT", bufs=3) as xp,
            tc.tile_pool(name="hT", bufs=3) as hp,
            tc.tile_pool(name="AT", bufs=3) as atp,
            tc.tile_pool(name="yT", bufs=2) as yp,
            tc.tile_pool(name="hps", bufs=2, space="PSUM") as hps,
            tc.tile_pool(name="cps", bufs=2, space="PSUM") as cps,
        ):
            # ---- constants ----
            W1s = cpool.tile([P, 2, U], F16, tag="W1s")
            nc.sync.dma_start(W1s, W1_d.ap())
            W2s = cpool.tile([P, 2, U], F16, tag="W2s")
            nc.sync.dma_start(W2s, W2_d.ap())
            b1s = cpool.tile([P, 2], F32, tag="b1s")
            nc.sync.dma_start(b1s, b1_d.ap())
            b2s = cpool.tile([1, U], F16, tag="b2s")
            nc.gpsimd.dma_start(b2s, b2_d.ap())
            ones_t = cpool.tile([1, TB], F16, tag="ones")
            nc.sync.dma_start(ones_t, ones_d.ap())

            st = [dict() for _ in range(NG)]

            # ---- background work-item constructors ----
            def xin(g):
                def run():
                    s = st[g]
                    s["xT"] = xp.tile([P, 2, TB], F16, tag="xT", name="xT")
                    nc.sync.dma_start(s["xT"], xT_d.ap()[g])
                return run

            def h_mm(g, uc, dc):
                def run():
                    s = st[g]
                    if uc == 0 and dc == 0:
                        s["hp"] = hps.tile([P, 2, TB], F32, tag="hp", name="hp")
                    nc.tensor.matmul(
                        s["hp"][:, uc, :],
                        W1s[:, dc, uc * P:(uc + 1) * P],
                        s["xT"][:, dc, :],
                        start=(dc == 0), stop=(dc == 1))
                return run

            def h_act(g, uc, half, nchunks=2):
                HH = TB // nchunks
                def run():
                    s = st[g]
                    if uc == 0 and half == 0:
                        s["hT"] = hp.tile([P, 2, TB], F16, tag="hT", name="hT")
                    sl = slice(half * HH, (half + 1) * HH)
                    if use_b1:
                        nc.scalar.activation(
                            s["hT"][:, uc, sl], s["hp"][:, uc, sl],
                            mybir.ActivationFunctionType.Tanh,
                            bias=b1s[:, uc:uc + 1])
                    else:
                        nc.scalar.activation(
                            s["hT"][:, uc, sl], s["hp"][:, uc, sl],
                            mybir.ActivationFunctionType.Tanh)
                return run

            def c_ones(g, mc, cols=None):
                # b2 broadcast; with cols=BS used only as group-0 col-0 init
                def run():
                    s = st[g]
                    if mc == 0:
                        s["cp"] = cps.tile([P, 2, TB], F32, tag="cp", name="cp")
                    if cols is None:
                        nc.tensor.matmul(
                            s["cp"][:, mc, :], b2s[:, mc * P:(mc + 1) * P],
                            ones_t, start=True, stop=False)
                    else:
                        nc.tensor.matmul(
                            s["cp"][:, mc, 0:cols], b2s[:, mc * P:(mc + 1) * P],
                            ones_t[:, 0:cols], start=True, stop=False)
                return run

            def c_j0(g, mc, kc):
                # C col 0 of group g needs h of the last step of group g-1.
                # Without the b2 ones-matmul this is the bank's first matmul:
                # kc==0 carries start=True (pending-zero init of the bank).
                def run():
                    nc.tensor.matmul(
                        st[g]["cp"][:, mc, 0:BS],
                        W2s[:, kc, mc * P:(mc + 1) * P],
                        st[g - 1]["hT"][:, kc, TB - BS:TB],
                        start=(not use_b2 and kc == 0), stop=False)
                return run

            def c_j0_alloc(g):
                def run():
                    st[g]["cp"] = cps.tile([P, 2, TB], F32, tag="cp", name="cp")
                return run

            def c_big(g, mc, kc):
                # C cols 32:512 of group g from h cols 0:480 of group g
                def run():
                    nc.tensor.matmul(
                        st[g]["cp"][:, mc, BS:TB],
                        W2s[:, kc, mc * P:(mc + 1) * P],
                        st[g]["hT"][:, kc, 0:TB - BS],
                        start=False, stop=False)
                return run

            def yout(g):
                def run():
                    nc.gpsimd.dma_start(y_d.ap()[g], st[g]["yT"])
                return run

            def group_bg(g):
                """Work items queued at j==0 of scan group g."""
                items = []
                if g - 1 >= 0:
                    items.append(("dma", 1, yout(g - 1)))
                if g + 2 < NG:
                    items.append(("dma", 1, xin(g + 2)))
                if g + 1 < NG:
                    for uc in (0, 1):
                        for dc in (0, 1):
                            items.append(("pe", C_HMM, h_mm(g + 1, uc, dc)))
                    for uc in (0, 1):
                        for half in (0, 1):
                            items.append(("act", C_HACT, h_act(g + 1, uc, half)))
                    if use_b2:
                        for mc in (0, 1):
                            items.append(("pe", C_ONES, c_ones(g + 1, mc)))
                    else:
                        items.append(("pe", 1, c_j0_alloc(g + 1)))
                    for mc in (0, 1):
                        for kc in (0, 1):
                            items.append(("pe", C_J0, c_j0(g + 1, mc, kc)))
                    for mc in (0, 1):
                        for kc in (0, 1):
                            items.append(("pe", C_BIG, c_big(g + 1, mc, kc)))
                return items

            # ---- serial scan step ----
            AT_prev = [None]

            def scan_step(t):
                g, j = divmod(t, GR)
                s = st[g]
                cp = s["cp"]
                if t > 0:
                    for mc in (0, 1):
                        for kc in (0, 1):
                            nc.tensor.matmul(
                                cp[:, mc, j * BS:(j + 1) * BS],
                                W2s[:, kc, mc * P:(mc + 1) * P],
                                AT_prev[0][:, kc, :],
                                start=False,
                                stop=(j == GR - 1 and kc == 1))
                AT = atp.tile([P, 2, BS], F16, tag="AT", name="AT")
                nc.scalar.activation(
                    AT, cp[:, :, j * BS:(j + 1) * BS],
                    mybir.ActivationFunctionType.Tanh)
                AT_prev[0] = AT
                if j == 0:
                    s["yT"] = yp.tile([P, 2, TB], F32, tag="yT", name="yT")
                nc.vector.tensor_add(
                    out=s["yT"][:, :, j * BS:(j + 1) * BS],
                    in0=s["hT"][:, :, j * BS:(j + 1) * BS],
                    in1=AT)

            # ---- prologue: group 0 fully prepared before the scan ----
            for g0 in range(min(2, NG)):
                xin(g0)()
            for uc in (0, 1):
                for dc in (0, 1):
                    h_mm(0, uc, dc)()
            for uc in (0, 1):
                for chunk in range(2):
                    h_act(0, uc, chunk, nchunks=2)()
            if use_b2:
                for mc in (0, 1):
                    c_ones(0, mc)()
            else:
                # col-0 init for group 0 (z_0 = b2 = 0)
                for mc in (0, 1):
                    c_ones(0, mc, cols=BS)()
            for mc in (0, 1):
                for kc in (0, 1):
                    c_big(0, mc, kc)()

            # ---- scan with budgeted background drain ----
            from collections import deque
            work = deque()
            pe_cr = act_cr = dma_cr = 0.0
            for t in range(T_steps):
                g, j = divmod(t, GR)
                if j == 0:
                    work.extend(group_bg(g))
                scan_step(t)
                pe_cr = min(pe_cr + PE_RATE, PE_CAP)
                act_cr = min(act_cr + ACT_RATE, ACT_CAP)
                dma_cr = min(dma_cr + 1, 2)
                while work:
                    kind, cost, run = work[0]
                    if kind == "pe":
                        if pe_cr < cost:
                            break
                        pe_cr -= cost
                    elif kind == "act":
                        if act_cr < cost:
                            break
                        act_cr -= cost
                    else:
                        if dma_cr < cost:
                            break
                        dma_cr -= cost
                    work.popleft()
                    run()
            while work:
                work.popleft()[2]()
            yout(NG - 1)()

    nc.finalize()
    return nc


_NC_CACHE = {}


def _get_nc(T_steps=T, use_b2=True, use_b1=True):
    key = (T_steps, use_b1, use_b2)
    if key not in _NC_CACHE:
        _NC_CACHE[key] = build_rnn(T_steps, use_b1=use_b1, use_b2=use_b2)
    return _NC_CACHE[key]


def kernel(x, W1, b1, W2, b2):
    Tn = x.shape[1]
    NG = Tn // GR

    x = np.asarray(x, dtype=np.float32)
    W1 = np.asarray(W1, dtype=np.float32)
    b1 = np.asarray(b1, dtype=np.float32)
    W2 = np.asarray(W2, dtype=np.float32)
    b2 = np.asarray(b2, dtype=np.float32)

    use_b1 = bool(np.any(b1))
    use_b2 = bool(np.any(b2))
    nc = _get_nc(Tn, use_b2=use_b2, use_b1=use_b1)

    # host-side pre-transposes (device layouts are partition-major)
    W1t = np.ascontiguousarray(
        W1.reshape(2, P, U).transpose(1, 0, 2)).astype(np.float16)
    W2t = np.ascontiguousarray(
        W2.reshape(2, P, U).transpose(1, 0, 2)).astype(np.float16)
    b1t = np.ascontiguousarray(b1.reshape(2, P).T)
    b2t = b2.reshape(1, U).astype(np.float16)

    in_maps = []
    for c in range(NCORES):
        xc = x[c * BS:(c + 1) * BS]  # [BS, T, D]
        # xT[g, p, dc, j*BS + b] = x[b, g*GR+j, dc*P + p]
        xt = xc.reshape(BS, NG, GR, 2, P).transpose(1, 4, 3, 2, 0)
        xt = np.ascontiguousarray(xt).astype(np.float16).reshape(NG, P, 2, TB)
        in_maps.append({
            "xT": xt, "W1t": W1t, "b1t": b1t, "W2t": W2t, "b2t": b2t,
        })
    res = bass_utils.run_bass_kernel_spmd(nc, in_maps, core_ids=list(range(NCORES)))

    out = np.empty((B, Tn, U), dtype=np.float32)
    for c in range(NCORES):
        yt = res.results[c]["yT"]  # [NG, P, 2, TB]
        # y[b, g*GR+j, mc*P + p] = yT[g, p, mc, j*BS + b]
        yc = yt.reshape(NG, P, 2, GR, BS).transpose(4, 0, 3, 2, 1)
        out[c * BS:(c + 1) * BS] = yc.reshape(BS, Tn, U)
    return out


# revision 24
# speedup vs baseline: 1.0140x; 1.0140x over previous
"""Trainium2 Bass kernel for CustomRNN:
    h = tanh(x @ W1 + b1)                         [B,T,U]
    y_t = h_t + tanh(y_{t-1} @ W2 + b2)           (scan over T, y_{-1} = 0)

Strategy (8 NeuronCores, data-parallel over batch B=256 -> 32 rows/core):
  * All device-side layouts are transposed ([u/d on partitions, (step, batch)
    on free]) and the host does the transposes with numpy before/after the
    device call, so the kernel has zero on-chip transposes and every DMA has
    a 128-wide leading dim (the sim's DMA cost is bytes / leading-dim).
  * Split state: y_t = h_t + A_t with A_t = tanh(z_t),
    z_t = b2 + h_{t-1}@W2 + A_{t-1}@W2.
    The b2 term and h@W2 terms are batched into background GEMMs that
    deposit C_t = b2 + h_{t-1}@W2 for a whole 16-step group directly into a
    PSUM bank pair; the serial scan step is then only:
        4 small matmuls (A_{t-1}@W2, start=False accumulate onto C) -> tanh
    i.e. one PE->ACT->PE round trip per step; everything else (input
    projection GEMM + tanh, C GEMMs, y = h + A adds, DMA in/out) runs in
    engine-idle windows via a cost-budgeted background work queue.
  * When b1/b2 are all-zero (they are for this problem's inputs) the bias
    ones-matmuls are dropped; PSUM zero-regions are then initialized by the
    first C matmul (start=True marks the bank pending-zero, so later
    accumulating matmuls first-touch-overwrite).
  * f16 everywhere on-chip except PSUM accumulation (f32) and y (f32).
"""

import numpy as np

import concourse.bacc as bacc
import concourse.bass as bass
import concourse.mybir as mybir
import concourse.tile as tile
from concourse import bass_utils

F32 = mybir.dt.float32
F16 = mybir.dt.float16

B, T, D, U = 256, 512, 256, 256
NCORES = 8
BS = B // NCORES   # 32 batch rows per core
GR = 16            # scan steps per group
TB = GR * BS       # free columns per group (512), col = j*BS + b
P = 128

# background work-item budget costs (ns; ~2x actual so the drain paces at
# most one big PE op into each chain idle window)
C_HMM, C_ONES, C_J0, C_BIG, C_HACT = 427, 427, 30, 402, 398
PE_RATE, PE_CAP = 460, 1400
HACT_N = 8
ACT_RATE, ACT_CAP = 420, 840


def build_rnn(T_steps=T, use_b1=True, use_b2=True):
    assert T_steps % GR == 0
    NG = T_steps // GR

    nc = bacc.Bacc("TRN2", debug=False)

    xT_d = nc.dram_tensor("xT", (NG, P, 2, TB), F16, kind="ExternalInput")
    W1_d = nc.dram_tensor("W1t", (P, 2, U), F16, kind="ExternalInput")
    b1_d = nc.dram_tensor("b1t", (P, 2), F32, kind="ExternalInput")
    W2_d = nc.dram_tensor("W2t", (P, 2, U), F16, kind="ExternalInput")
    b2_d = nc.dram_tensor("b2t", (1, U), F16, kind="ExternalInput")
    y_d = nc.dram_tensor("yT", (NG, P, 2, TB), F32, kind="ExternalOutput")
    ones_d = nc.inline_tensor(np.ones((1, TB), dtype=np.float16), "ones_row")

    with tile.TileContext(nc) as tc:
        with (
            tc.tile_pool(name="const", bufs=1) as cpool,
            tc.tile_pool(name="xT", bufs=3) as xp,
            tc.tile_pool(name="hT", bufs=3) as hp,
            tc.tile_pool(name="AT", bufs=3) as atp,
            tc.tile_pool(name="yT", bufs=2) as yp,
            tc.tile_pool(name="hps", bufs=2, space="PSUM") as hps,
            tc.tile_pool(name="cps", bufs=2, space="PSUM") as cps,
        ):
            # ---- constants ----
            W1s = cpool.tile([P, 2, U], F16, tag="W1s")
            nc.sync.dma_start(W1s, W1_d.ap())
            W2s = cpool.tile([P, 2, U], F16, tag="W2s")
            nc.sync.dma_start(W2s, W2_d.ap())
            b1s = cpool.tile([P, 2], F32, tag="b1s")
            nc.sync.dma_start(b1s, b1_d.ap())
            b2s = cpool.tile([1, U], F16, tag="b2s")
            nc.gpsimd.dma_start(b2s, b2_d.ap())
            ones_t = cpool.tile([1, TB], F16, tag="ones")
            nc.sync.dma_start(ones_t, ones_d.ap())

            st = [dict() for _ in range(NG)]

            # ---- background work-item constructors ----
            def xin(g):
                def run():
                    s = st[g]
                    s["xT"] = xp.tile([P, 2, TB], F16, tag="xT", name="xT")
                    nc.sync.dma_start(s["xT"], xT_d.ap()[g])
                return run

            def h_mm(g, uc, dc):
                def run():
                    s = st[g]
                    if uc == 0 and dc == 0:
                        s["hp"] = hps.tile([P, 2, TB], F32, tag="hp", name="hp")
                    nc.tensor.matmul(
                        s["hp"][:, uc, :],
                        W1s[:, dc, uc * P:(uc + 1) * P],
                        s["xT"][:, dc, :],
                        start=(dc == 0), stop=(dc == 1))
                return run

            def h_act(g, uc, half, nchunks=2):
                HH = TB // nchunks
                def run():
                    s = st[g]
                    if uc == 0 and half == 0:
                        s["hT"] = hp.tile([P, 2, TB], F16, tag="hT", name="hT")
                    sl = slice(half * HH, (half + 1) * HH)
                    if use_b1:
                        nc.scalar.activation(
                            s["hT"][:, uc, sl], s["hp"][:, uc, sl],
                            mybir.ActivationFunctionType.Tanh,
                            bias=b1s[:, uc:uc + 1])
                    else:
                        nc.scalar.activation(
                            s["hT"][:, uc, sl], s["hp"][:, uc, sl],
                            mybir.ActivationFunctionType.Tanh)
                return run

            def c_ones(g, mc, cols=None):
                # b2 broadcast; with cols=BS used only as group-0 col-0 init
                def run():
                    s = st[g]
                    if mc == 0:
                        s["cp"] = cps.tile([P, 2, TB], F32, tag="cp", name="cp")
                    if cols is None:
                        nc.tensor.matmul(
                            s["cp"][:, mc, :], b2s[:, mc * P:(mc + 1) * P],
                            ones_t, start=True, stop=False)
                    else:
                        nc.tensor.matmul(
                            s["cp"][:, mc, 0:cols], b2s[:, mc * P:(mc + 1) * P],
                            ones_t[:, 0:cols], start=True, stop=False)
                return run

            def c_j0(g, mc, kc):
                # C col 0 of group g needs h of the last step of group g-1.
                # Without the b2 ones-matmul this is the bank's first matmul:
                # kc==0 carries start=True (pending-zero init of the bank).
                def run():
                    nc.tensor.matmul(
                        st[g]["cp"][:, mc, 0:BS],
                        W2s[:, kc, mc * P:(mc + 1) * P],
                        st[g - 1]["hT"][:, kc, TB - BS:TB],
                        start=(not use_b2 and kc == 0), stop=False)
                return run

            def c_j0_alloc(g):
                def run():
                    st[g]["cp"] = cps.tile([P, 2, TB], F32, tag="cp", name="cp")
                return run

            def c_big(g, mc, kc):
                # C cols 32:512 of group g from h cols 0:480 of group g
                def run():
                    nc.tensor.matmul(
                        st[g]["cp"][:, mc, BS:TB],
                        W2s[:, kc, mc * P:(mc + 1) * P],
                        st[g]["hT"][:, kc, 0:TB - BS],
                        start=False, stop=False)
                return run

            def yout(g):
                def run():
                    nc.gpsimd.dma_start(y_d.ap()[g], st[g]["yT"])
                return run

            def group_bg(g):
                """Work items queued at j==0 of scan group g."""
                items = []
                if g - 1 >= 0:
                    items.append(("dma", 1, yout(g - 1)))
                if g + 2 < NG:
                    items.append(("dma", 1, xin(g + 2)))
                if g + 1 < NG:
                    for uc in (0, 1):
                        for dc in (0, 1):
                            items.append(("pe", C_HMM, h_mm(g + 1, uc, dc)))
                    for uc in (0, 1):
                        for q in range(HACT_N):
                            items.append(
                                ("act", int(TB / HACT_N * 0.833) + 185,
                                 h_act(g + 1, uc, q, nchunks=HACT_N)))
                    if use_b2:
                        for mc in (0, 1):
                            items.append(("pe", C_ONES, c_ones(g + 1, mc)))
                    else:
                        items.append(("pe", 1, c_j0_alloc(g + 1)))
                    for mc in (0, 1):
                        for kc in (0, 1):
                            items.append(("pe", C_J0, c_j0(g + 1, mc, kc)))
                    for mc in (0, 1):
                        for kc in (0, 1):
                            items.append(("pe", C_BIG, c_big(g + 1, mc, kc)))
                return items

            # ---- serial scan step ----
            AT_prev = [None]

            def scan_step(t):
                g, j = divmod(t, GR)
                s = st[g]
                cp = s["cp"]
                if t > 0:
                    for mc in (0, 1):
                        for kc in (0, 1):
                            nc.tensor.matmul(
                                cp[:, mc, j * BS:(j + 1) * BS],
                                W2s[:, kc, mc * P:(mc + 1) * P],
                                AT_prev[0][:, kc, :],
                                start=False,
                                stop=(j == GR - 1 and kc == 1))
                AT = atp.tile([P, 2, BS], F16, tag="AT", name="AT")
                nc.scalar.activation(
                    AT, cp[:, :, j * BS:(j + 1) * BS],
                    mybir.ActivationFunctionType.Tanh)
                AT_prev[0] = AT
                if j == 0:
                    s["yT"] = yp.tile([P, 2, TB], F32, tag="yT", name="yT")
                nc.vector.tensor_add(
                    out=s["yT"][:, :, j * BS:(j + 1) * BS],
                    in0=s["hT"][:, :, j * BS:(j + 1) * BS],
                    in1=AT)

            # ---- prologue: group 0 fully prepared before the scan ----
            for g0 in range(min(2, NG)):
                xin(g0)()
            for uc in (0, 1):
                for dc in (0, 1):
                    h_mm(0, uc, dc)()
            for uc in (0, 1):
                for chunk in range(2):
                    h_act(0, uc, chunk, nchunks=2)()
            if use_b2:
                for mc in (0, 1):
                    c_ones(0, mc)()
            else:
                # col-0 init for group 0 (z_0 = b2 = 0)
                for mc in (0, 1):
                    c_ones(0, mc, cols=BS)()
            for mc in (0, 1):
                for kc in (0, 1):
                    c_big(0, mc, kc)()

            # ---- scan with budgeted background drain ----
            from collections import deque
            work = deque()
            pe_cr = act_cr = dma_cr = 0.0
            for t in range(T_steps):
                g, j = divmod(t, GR)
                if j == 0:
                    work.extend(group_bg(g))
                scan_step(t)
                pe_cr = min(pe_cr + PE_RATE, PE_CAP)
                act_cr = min(act_cr + ACT_RATE, ACT_CAP)
                dma_cr = min(dma_cr + 1, 2)
                while work:
                    kind, cost, run = work[0]
                    if kind == "pe":
                        if pe_cr < cost:
                            break
                        pe_cr -= cost
                    elif kind == "act":
                        if act_cr < cost:
                            break
                        act_cr -= cost
                    else:
                        if dma_cr < cost:
                            break
                        dma_cr -= cost
                    work.popleft()
                    run()
            while work:
                work.popleft()[2]()
            yout(NG - 1)()

    nc.finalize()
    return nc


_NC_CACHE = {}


def _get_nc(T_steps=T, use_b2=True, use_b1=True):
    key = (T_steps, use_b1, use_b2)
    if key not in _NC_CACHE:
        _NC_CACHE[key] = build_rnn(T_steps, use_b1=use_b1, use_b2=use_b2)
    return _NC_CACHE[key]


def kernel(x, W1, b1, W2, b2):
    Tn = x.shape[1]
    NG = Tn // GR

    x = np.asarray(x, dtype=np.float32)
    W1 = np.asarray(W1, dtype=np.float32)
    b1 = np.asarray(b1, dtype=np.float32)
    W2 = np.asarray(W2, dtype=np.float32)
    b2 = np.asarray(b2, dtype=np.float32)

    use_b1 = bool(np.any(b1))
    use_b2 = bool(np.any(b2))
    nc = _get_nc(Tn, use_b2=use_b2, use_b1=use_b1)

    # host-side pre-transposes (device layouts are partition-major)
    W1t = np.ascontiguousarray(
        W1.reshape(2, P, U).transpose(1, 0, 2)).astype(np.float16)
    W2t = np.ascontiguousarray(
        W2.reshape(2, P, U).transpose(1, 0, 2)).astype(np.float16)
    b1t = np.ascontiguousarray(b1.reshape(2, P).T)
    b2t = b2.reshape(1, U).astype(np.float16)

    in_maps = []
    for c in range(NCORES):
        xc = x[c * BS:(c + 1) * BS]  # [BS, T, D]
        # xT[g, p, dc, j*BS + b] = x[b, g*GR+j, dc*P + p]
        xt = xc.reshape(BS, NG, GR, 2, P).transpose(1, 4, 3, 2, 0)
        xt = np.ascontiguousarray(xt).astype(np.float16).reshape(NG, P, 2, TB)
        in_maps.append({
            "xT": xt, "W1t": W1t, "b1t": b1t, "W2t": W2t, "b2t": b2t,
        })
    res = bass_utils.run_bass_kernel_spmd(nc, in_maps, core_ids=list(range(NCORES)))

    out = np.empty((B, Tn, U), dtype=np.float32)
    for c in range(NCORES):
        yt = res.results[c]["yT"]  # [NG, P, 2, TB]
        # y[b, g*GR+j, mc*P + p] = yT[g, p, mc, j*BS + b]
        yc = yt.reshape(NG, P, 2, GR, BS).transpose(4, 0, 3, 2, 1)
        out[c * BS:(c + 1) * BS] = yc.reshape(BS, Tn, U)
    return out


# revision 26
# speedup vs baseline: 1.0287x; 1.0146x over previous
"""Trainium2 Bass kernel for CustomRNN:
    h = tanh(x @ W1 + b1)                         [B,T,U]
    y_t = h_t + tanh(y_{t-1} @ W2 + b2)           (scan over T, y_{-1} = 0)

Strategy (8 NeuronCores, data-parallel over batch B=256 -> 32 rows/core):
  * All device-side layouts are transposed ([u/d on partitions, (step, batch)
    on free]) and the host does the transposes with numpy before/after the
    device call, so the kernel has zero on-chip transposes and every DMA has
    a 128-wide leading dim (the sim's DMA cost is bytes / leading-dim).
  * Split state: y_t = h_t + A_t with A_t = tanh(z_t),
    z_t = b2 + h_{t-1}@W2 + A_{t-1}@W2.
    The b2 term and h@W2 terms are batched into background GEMMs that
    deposit C_t = b2 + h_{t-1}@W2 for a whole 16-step group directly into a
    PSUM bank pair; the serial scan step is then only:
        4 small matmuls (A_{t-1}@W2, start=False accumulate onto C) -> tanh
    i.e. one PE->ACT->PE round trip per step; everything else (input
    projection GEMM + tanh, C GEMMs, y = h + A adds, DMA in/out) runs in
    engine-idle windows via a cost-budgeted background work queue.
  * When b1/b2 are all-zero (they are for this problem's inputs) the bias
    ones-matmuls are dropped; PSUM zero-regions are then initialized by the
    first C matmul (start=True marks the bank pending-zero, so later
    accumulating matmuls first-touch-overwrite).
  * f16 everywhere on-chip except PSUM accumulation (f32) and y (f32).
"""

import numpy as np

import concourse.bacc as bacc
import concourse.bass as bass
import concourse.mybir as mybir
import concourse.tile as tile
from concourse import bass_utils

F32 = mybir.dt.float32
F16 = mybir.dt.float16

B, T, D, U = 256, 512, 256, 256
NCORES = 8
BS = B // NCORES   # 32 batch rows per core
GR = 16            # scan steps per group
TB = GR * BS       # free columns per group (512), col = j*BS + b
P = 128

# background work-item budget costs (ns; ~2x actual so the drain paces at
# most one big PE op into each chain idle window)
C_HMM, C_ONES, C_J0, C_BIG, C_HACT = 427, 427, 30, 402, 398
PE_RATE, PE_CAP = 460, 1400
HACT_N = 8
# h tanh column chunks per u-plane per group: sized so each insertion
# (cols*0.833+185 ns) nearly fits the chain's ~213ns ACT idle window while
# keeping total ACT busy at ~the chain latency (empirically tuned)
HACT_SPLIT = (160, 64, 64, 64, 64, 64, 32)
ACT_RATE, ACT_CAP = 420, 840


def build_rnn(T_steps=T, use_b1=True, use_b2=True):
    assert T_steps % GR == 0
    NG = T_steps // GR

    nc = bacc.Bacc("TRN2", debug=False)

    xT_d = nc.dram_tensor("xT", (NG, P, 2, TB), F16, kind="ExternalInput")
    W1_d = nc.dram_tensor("W1t", (P, 2, U), F16, kind="ExternalInput")
    b1_d = nc.dram_tensor("b1t", (P, 2), F32, kind="ExternalInput")
    W2_d = nc.dram_tensor("W2t", (P, 2, U), F16, kind="ExternalInput")
    b2_d = nc.dram_tensor("b2t", (1, U), F16, kind="ExternalInput")
    y_d = nc.dram_tensor("yT", (NG, P, 2, TB), F32, kind="ExternalOutput")
    ones_d = nc.inline_tensor(np.ones((1, TB), dtype=np.float16), "ones_row")

    with tile.TileContext(nc) as tc:
        with (
            tc.tile_pool(name="const", bufs=1) as cpool,
            tc.tile_pool(name="xT", bufs=3) as xp,
            tc.tile_pool(name="hT", bufs=3) as hp,
            tc.tile_pool(name="AT", bufs=3) as atp,
            tc.tile_pool(name="yT", bufs=2) as yp,
            tc.tile_pool(name="hps", bufs=2, space="PSUM") as hps,
            tc.tile_pool(name="cps", bufs=2, space="PSUM") as cps,
        ):
            # ---- constants ----
            W1s = cpool.tile([P, 2, U], F16, tag="W1s")
            nc.sync.dma_start(W1s, W1_d.ap())
            W2s = cpool.tile([P, 2, U], F16, tag="W2s")
            nc.sync.dma_start(W2s, W2_d.ap())
            b1s = cpool.tile([P, 2], F32, tag="b1s")
            nc.sync.dma_start(b1s, b1_d.ap())
            b2s = cpool.tile([1, U], F16, tag="b2s")
            nc.gpsimd.dma_start(b2s, b2_d.ap())
            ones_t = cpool.tile([1, TB], F16, tag="ones")
            nc.sync.dma_start(ones_t, ones_d.ap())

            st = [dict() for _ in range(NG)]

            # ---- background work-item constructors ----
            def xin(g):
                def run():
                    s = st[g]
                    s["xT"] = xp.tile([P, 2, TB], F16, tag="xT", name="xT")
                    nc.sync.dma_start(s["xT"], xT_d.ap()[g])
                return run

            def h_mm(g, uc, dc):
                def run():
                    s = st[g]
                    if uc == 0 and dc == 0:
                        s["hp"] = hps.tile([P, 2, TB], F32, tag="hp", name="hp")
                    nc.tensor.matmul(
                        s["hp"][:, uc, :],
                        W1s[:, dc, uc * P:(uc + 1) * P],
                        s["xT"][:, dc, :],
                        start=(dc == 0), stop=(dc == 1))
                return run

            def h_act(g, uc, half, nchunks=2, sl=None):
                HH = TB // nchunks
                def run():
                    s = st[g]
                    if uc == 0 and (half == 0 or half is None):
                        s["hT"] = hp.tile([P, 2, TB], F16, tag="hT", name="hT")
                    _sl = sl if sl is not None else slice(half * HH, (half + 1) * HH)
                    if use_b1:
                        nc.scalar.activation(
                            s["hT"][:, uc, _sl], s["hp"][:, uc, _sl],
                            mybir.ActivationFunctionType.Tanh,
                            bias=b1s[:, uc:uc + 1])
                    else:
                        nc.scalar.activation(
                            s["hT"][:, uc, _sl], s["hp"][:, uc, _sl],
                            mybir.ActivationFunctionType.Tanh)
                return run

            def c_ones(g, mc, cols=None):
                # b2 broadcast; with cols=BS used only as group-0 col-0 init
                def run():
                    s = st[g]
                    if mc == 0:
                        s["cp"] = cps.tile([P, 2, TB], F32, tag="cp", name="cp")
                    if cols is None:
                        nc.tensor.matmul(
                            s["cp"][:, mc, :], b2s[:, mc * P:(mc + 1) * P],
                            ones_t, start=True, stop=False)
                    else:
                        nc.tensor.matmul(
                            s["cp"][:, mc, 0:cols], b2s[:, mc * P:(mc + 1) * P],
                            ones_t[:, 0:cols], start=True, stop=False)
                return run

            def c_j0(g, mc, kc):
                # C col 0 of group g needs h of the last step of group g-1.
                # Without the b2 ones-matmul this is the bank's first matmul:
                # kc==0 carries start=True (pending-zero init of the bank).
                def run():
                    nc.tensor.matmul(
                        st[g]["cp"][:, mc, 0:BS],
                        W2s[:, kc, mc * P:(mc + 1) * P],
                        st[g - 1]["hT"][:, kc, TB - BS:TB],
                        start=(not use_b2 and kc == 0), stop=False)
                return run

            def c_j0_alloc(g):
                def run():
                    st[g]["cp"] = cps.tile([P, 2, TB], F32, tag="cp", name="cp")
                return run

            def c_big(g, mc, kc):
                # C cols 32:512 of group g from h cols 0:480 of group g
                def run():
                    nc.tensor.matmul(
                        st[g]["cp"][:, mc, BS:TB],
                        W2s[:, kc, mc * P:(mc + 1) * P],
                        st[g]["hT"][:, kc, 0:TB - BS],
                        start=False, stop=False)
                return run

            def yout(g):
                def run():
                    nc.gpsimd.dma_start(y_d.ap()[g], st[g]["yT"])
                return run

            def group_bg(g):
                """Work items queued at j==0 of scan group g."""
                items = []
                if g - 1 >= 0:
                    items.append(("dma", 1, yout(g - 1)))
                if g + 2 < NG:
                    items.append(("dma", 1, xin(g + 2)))
                if g + 1 < NG:
                    for uc in (0, 1):
                        for dc in (0, 1):
                            items.append(("pe", C_HMM, h_mm(g + 1, uc, dc)))
                    for uc in (0, 1):
                        col = 0
                        for qi, cols in enumerate(HACT_SPLIT):
                            items.append(
                                ("act", int(cols * 0.833) + 185,
                                 h_act(g + 1, uc, qi if col == 0 else qi,
                                       sl=slice(col, col + cols))))
                            col += cols
                        assert col == TB
                    if use_b2:
                        for mc in (0, 1):
                            items.append(("pe", C_ONES, c_ones(g + 1, mc)))
                    else:
                        items.append(("pe", 1, c_j0_alloc(g + 1)))
                    for mc in (0, 1):
                        for kc in (0, 1):
                            items.append(("pe", C_J0, c_j0(g + 1, mc, kc)))
                    for mc in (0, 1):
                        for kc in (0, 1):
                            items.append(("pe", C_BIG, c_big(g + 1, mc, kc)))
                return items

            # ---- serial scan step ----
            AT_prev = [None]

            def scan_step(t):
                g, j = divmod(t, GR)
                s = st[g]
                cp = s["cp"]
                if t > 0:
                    for mc in (0, 1):
                        for kc in (0, 1):
                            nc.tensor.matmul(
                                cp[:, mc, j * BS:(j + 1) * BS],
                                W2s[:, kc, mc * P:(mc + 1) * P],
                                AT_prev[0][:, kc, :],
                                start=False,
                                stop=(j == GR - 1 and kc == 1))
                AT = atp.tile([P, 2, BS], F16, tag="AT", name="AT")
                nc.scalar.activation(
                    AT, cp[:, :, j * BS:(j + 1) * BS],
                    mybir.ActivationFunctionType.Tanh)
                AT_prev[0] = AT
                if j == 0:
                    s["yT"] = yp.tile([P, 2, TB], F32, tag="yT", name="yT")
                nc.vector.tensor_add(
                    out=s["yT"][:, :, j * BS:(j + 1) * BS],
                    in0=s["hT"][:, :, j * BS:(j + 1) * BS],
                    in1=AT)

            # ---- prologue: group 0 fully prepared before the scan ----
            for g0 in range(min(2, NG)):
                xin(g0)()
            for uc in (0, 1):
                for dc in (0, 1):
                    h_mm(0, uc, dc)()
            for uc in (0, 1):
                for chunk in range(2):
                    h_act(0, uc, chunk, nchunks=2)()
            if use_b2:
                for mc in (0, 1):
                    c_ones(0, mc)()
            else:
                # col-0 init for group 0 (z_0 = b2 = 0)
                for mc in (0, 1):
                    c_ones(0, mc, cols=BS)()
            for mc in (0, 1):
                for kc in (0, 1):
                    c_big(0, mc, kc)()

            # ---- scan with budgeted background drain ----
            from collections import deque
            work = deque()
            pe_cr = act_cr = dma_cr = 0.0
            for t in range(T_steps):
                g, j = divmod(t, GR)
                if j == 0:
                    work.extend(group_bg(g))
                scan_step(t)
                pe_cr = min(pe_cr + PE_RATE, PE_CAP)
                act_cr = min(act_cr + ACT_RATE, ACT_CAP)
                dma_cr = min(dma_cr + 1, 2)
                while work:
                    kind, cost, run = work[0]
                    if kind == "pe":
                        if pe_cr < cost:
                            break
                        pe_cr -= cost
                    elif kind == "act":
                        if act_cr < cost:
                            break
                        act_cr -= cost
                    else:
                        if dma_cr < cost:
                            break
                        dma_cr -= cost
                    work.popleft()
                    run()
            while work:
                work.popleft()[2]()
            yout(NG - 1)()

    nc.finalize()
    return nc


_NC_CACHE = {}


def _get_nc(T_steps=T, use_b2=True, use_b1=True):
    key = (T_steps, use_b1, use_b2)
    if key not in _NC_CACHE:
        _NC_CACHE[key] = build_rnn(T_steps, use_b1=use_b1, use_b2=use_b2)
    return _NC_CACHE[key]


def kernel(x, W1, b1, W2, b2):
    Tn = x.shape[1]
    NG = Tn // GR

    x = np.asarray(x, dtype=np.float32)
    W1 = np.asarray(W1, dtype=np.float32)
    b1 = np.asarray(b1, dtype=np.float32)
    W2 = np.asarray(W2, dtype=np.float32)
    b2 = np.asarray(b2, dtype=np.float32)

    use_b1 = bool(np.any(b1))
    use_b2 = bool(np.any(b2))
    nc = _get_nc(Tn, use_b2=use_b2, use_b1=use_b1)

    # host-side pre-transposes (device layouts are partition-major)
    W1t = np.ascontiguousarray(
        W1.reshape(2, P, U).transpose(1, 0, 2)).astype(np.float16)
    W2t = np.ascontiguousarray(
        W2.reshape(2, P, U).transpose(1, 0, 2)).astype(np.float16)
    b1t = np.ascontiguousarray(b1.reshape(2, P).T)
    b2t = b2.reshape(1, U).astype(np.float16)

    in_maps = []
    for c in range(NCORES):
        xc = x[c * BS:(c + 1) * BS]  # [BS, T, D]
        # xT[g, p, dc, j*BS + b] = x[b, g*GR+j, dc*P + p]
        xt = xc.reshape(BS, NG, GR, 2, P).transpose(1, 4, 3, 2, 0)
        xt = np.ascontiguousarray(xt).astype(np.float16).reshape(NG, P, 2, TB)
        in_maps.append({
            "xT": xt, "W1t": W1t, "b1t": b1t, "W2t": W2t, "b2t": b2t,
        })
    res = bass_utils.run_bass_kernel_spmd(nc, in_maps, core_ids=list(range(NCORES)))

    out = np.empty((B, Tn, U), dtype=np.float32)
    for c in range(NCORES):
        yt = res.results[c]["yT"]  # [NG, P, 2, TB]
        # y[b, g*GR+j, mc*P + p] = yT[g, p, mc, j*BS + b]
        yc = yt.reshape(NG, P, 2, GR, BS).transpose(4, 0, 3, 2, 1)
        out[c * BS:(c + 1) * BS] = yc.reshape(BS, Tn, U)
    return out


# revision 29
# speedup vs baseline: 1.0413x; 1.0123x over previous
"""Trainium2 Bass kernel for CustomRNN:
    h = tanh(x @ W1 + b1)                         [B,T,U]
    y_t = h_t + tanh(y_{t-1} @ W2 + b2)           (scan over T, y_{-1} = 0)

Strategy (8 NeuronCores, data-parallel over batch B=256 -> 32 rows/core):
  * All device-side layouts are transposed ([u/d on partitions, (step, batch)
    on free]) and the host does the transposes with numpy before/after the
    device call, so the kernel has zero on-chip transposes and every DMA has
    a 128-wide leading dim (the sim's DMA cost is bytes / leading-dim).
  * Split state: y_t = h_t + A_t with A_t = tanh(z_t),
    z_t = b2 + h_{t-1}@W2 + A_{t-1}@W2.
    The b2 term and h@W2 terms are batched into background GEMMs that
    deposit C_t = b2 + h_{t-1}@W2 for a whole 16-step group directly into a
    PSUM bank pair; the serial scan step is then only:
        4 small matmuls (A_{t-1}@W2, start=False accumulate onto C) -> tanh
    i.e. one PE->ACT->PE round trip per step; everything else (input
    projection GEMM + tanh, C GEMMs, y = h + A adds, DMA in/out) runs in
    engine-idle windows via a cost-budgeted background work queue.
  * When b1/b2 are all-zero (they are for this problem's inputs) the bias
    ones-matmuls are dropped; PSUM zero-regions are then initialized by the
    first C matmul (start=True marks the bank pending-zero, so later
    accumulating matmuls first-touch-overwrite).
  * f16 everywhere on-chip except PSUM accumulation (f32) and y (f32).
"""

import numpy as np

import concourse.bacc as bacc
import concourse.bass as bass
import concourse.mybir as mybir
import concourse.tile as tile
from concourse import bass_utils

F32 = mybir.dt.float32
F16 = mybir.dt.float16

B, T, D, U = 256, 512, 256, 256
NCORES = 8
BS = B // NCORES   # 32 batch rows per core
GR = 16            # scan steps per group
TB = GR * BS       # free columns per group (512), col = j*BS + b
P = 128

# background work-item budget costs (ns; ~2x actual so the drain paces at
# most one big PE op into each chain idle window)
C_HMM, C_ONES, C_J0, C_BIG, C_HACT = 427, 427, 30, 402, 398
PE_RATE, PE_CAP = 460, 1400
HACT_N = 8
# h tanh column chunks per u-plane per group: sized so each insertion
# (cols*0.833+185 ns) nearly fits the chain's ~213ns ACT idle window while
# keeping total ACT busy at ~the chain latency (empirically tuned)
HACT_SPLIT = (160, 64, 64, 64, 64, 64, 32)
ACT_RATE, ACT_CAP = 420, 840


def build_rnn(T_steps=T, use_b1=True, use_b2=True):
    assert T_steps % GR == 0
    NG = T_steps // GR

    nc = bacc.Bacc("TRN2", debug=False)

    xT_d = nc.dram_tensor("xT", (NG, P, 2, TB), F16, kind="ExternalInput")
    W1_d = nc.dram_tensor("W1t", (P, 2, U), F16, kind="ExternalInput")
    b1_d = nc.dram_tensor("b1t", (P, 2), F32, kind="ExternalInput")
    W2_d = nc.dram_tensor("W2t", (P, 2, U), F16, kind="ExternalInput")
    b2_d = nc.dram_tensor("b2t", (1, U), F16, kind="ExternalInput")
    y_d = nc.dram_tensor("yT", (NG, P, 2, TB), F32, kind="ExternalOutput")
    ones_d = nc.inline_tensor(np.ones((1, TB), dtype=np.float16), "ones_row")

    with tile.TileContext(nc) as tc:
        with (
            tc.tile_pool(name="const", bufs=1) as cpool,
            tc.tile_pool(name="xT", bufs=3) as xp,
            tc.tile_pool(name="hT", bufs=3) as hp,
            tc.tile_pool(name="AT", bufs=3) as atp,
            tc.tile_pool(name="yT", bufs=2) as yp,
            tc.tile_pool(name="hps", bufs=2, space="PSUM") as hps,
            tc.tile_pool(name="cps", bufs=2, space="PSUM") as cps,
        ):
            # ---- constants (issued on Pool so the x stream owns SP) ----
            W1s = cpool.tile([P, 2, U], F16, tag="W1s")
            nc.gpsimd.dma_start(W1s, W1_d.ap())
            W2s = cpool.tile([P, 2, U], F16, tag="W2s")
            nc.gpsimd.dma_start(W2s, W2_d.ap())
            b1s = cpool.tile([P, 2], F32, tag="b1s")
            nc.gpsimd.dma_start(b1s, b1_d.ap())
            b2s = cpool.tile([1, U], F16, tag="b2s")
            nc.gpsimd.dma_start(b2s, b2_d.ap())
            ones_t = cpool.tile([1, TB], F16, tag="ones")
            nc.gpsimd.dma_start(ones_t, ones_d.ap())
            # pre-warm the ACT tanh table off the critical path
            warm = cpool.tile([1, 2], F16, tag="warm")
            nc.scalar.activation(warm, ones_t[:, 0:2],
                                 mybir.ActivationFunctionType.Tanh)

            st = [dict() for _ in range(NG)]

            # ---- background work-item constructors ----
            def xin(g):
                def run():
                    s = st[g]
                    s["xT"] = xp.tile([P, 2, TB], F16, tag="xT", name="xT")
                    nc.sync.dma_start(s["xT"], xT_d.ap()[g])
                return run

            def h_mm(g, uc, dc):
                def run():
                    s = st[g]
                    if uc == 0 and dc == 0:
                        s["hp"] = hps.tile([P, 2, TB], F32, tag="hp", name="hp")
                    nc.tensor.matmul(
                        s["hp"][:, uc, :],
                        W1s[:, dc, uc * P:(uc + 1) * P],
                        s["xT"][:, dc, :],
                        start=(dc == 0), stop=(dc == 1))
                return run

            def h_act(g, uc, half, nchunks=2, sl=None):
                HH = TB // nchunks
                def run():
                    s = st[g]
                    if uc == 0 and (half == 0 or half is None):
                        s["hT"] = hp.tile([P, 2, TB], F16, tag="hT", name="hT")
                    _sl = sl if sl is not None else slice(half * HH, (half + 1) * HH)
                    if use_b1:
                        nc.scalar.activation(
                            s["hT"][:, uc, _sl], s["hp"][:, uc, _sl],
                            mybir.ActivationFunctionType.Tanh,
                            bias=b1s[:, uc:uc + 1])
                    else:
                        nc.scalar.activation(
                            s["hT"][:, uc, _sl], s["hp"][:, uc, _sl],
                            mybir.ActivationFunctionType.Tanh)
                return run

            def c_ones(g, mc, cols=None):
                # b2 broadcast; with cols=BS used only as group-0 col-0 init
                def run():
                    s = st[g]
                    if mc == 0:
                        s["cp"] = cps.tile([P, 2, TB], F32, tag="cp", name="cp")
                    if cols is None:
                        nc.tensor.matmul(
                            s["cp"][:, mc, :], b2s[:, mc * P:(mc + 1) * P],
                            ones_t, start=True, stop=False)
                    else:
                        nc.tensor.matmul(
                            s["cp"][:, mc, 0:cols], b2s[:, mc * P:(mc + 1) * P],
                            ones_t[:, 0:cols], start=True, stop=False)
                return run

            def c_j0(g, mc, kc):
                # C col 0 of group g needs h of the last step of group g-1.
                # Without the b2 ones-matmul this is the bank's first matmul:
                # kc==0 carries start=True (pending-zero init of the bank).
                def run():
                    nc.tensor.matmul(
                        st[g]["cp"][:, mc, 0:BS],
                        W2s[:, kc, mc * P:(mc + 1) * P],
                        st[g - 1]["hT"][:, kc, TB - BS:TB],
                        start=(not use_b2 and kc == 0), stop=False)
                return run

            def c_j0_alloc(g):
                def run():
                    st[g]["cp"] = cps.tile([P, 2, TB], F32, tag="cp", name="cp")
                return run

            def c_big(g, mc, kc):
                # C cols 32:512 of group g from h cols 0:480 of group g
                def run():
                    nc.tensor.matmul(
                        st[g]["cp"][:, mc, BS:TB],
                        W2s[:, kc, mc * P:(mc + 1) * P],
                        st[g]["hT"][:, kc, 0:TB - BS],
                        start=False, stop=False)
                return run

            def yout(g):
                def run():
                    nc.gpsimd.dma_start(y_d.ap()[g], st[g]["yT"])
                return run

            def yout_part(g, q, nq=4):
                QQ = TB // nq
                def run():
                    nc.gpsimd.dma_start(
                        y_d.ap()[g][:, :, q * QQ:(q + 1) * QQ],
                        st[g]["yT"][:, :, q * QQ:(q + 1) * QQ])
                return run

            def group_bg(g):
                """Work items queued at j==0 of scan group g."""
                items = []
                if g - 1 >= 0:
                    items.append(("dma", 1, yout(g - 1)))
                if g + 2 < NG:
                    items.append(("dma", 1, xin(g + 2)))
                if g + 1 < NG:
                    for uc in (0, 1):
                        for dc in (0, 1):
                            items.append(("pe", C_HMM, h_mm(g + 1, uc, dc)))
                    for uc in (0, 1):
                        col = 0
                        for qi, cols in enumerate(HACT_SPLIT):
                            items.append(
                                ("act", int(cols * 0.833) + 185,
                                 h_act(g + 1, uc, qi if col == 0 else qi,
                                       sl=slice(col, col + cols))))
                            col += cols
                        assert col == TB
                    if use_b2:
                        for mc in (0, 1):
                            items.append(("pe", C_ONES, c_ones(g + 1, mc)))
                    else:
                        items.append(("pe", 1, c_j0_alloc(g + 1)))
                    for mc in (0, 1):
                        for kc in (0, 1):
                            items.append(("pe", C_J0, c_j0(g + 1, mc, kc)))
                    for mc in (0, 1):
                        for kc in (0, 1):
                            items.append(("pe", C_BIG, c_big(g + 1, mc, kc)))
                return items

            # ---- serial scan step ----
            AT_prev = [None]

            def scan_step(t):
                g, j = divmod(t, GR)
                s = st[g]
                cp = s["cp"]
                if t > 0:
                    for mc in (0, 1):
                        for kc in (0, 1):
                            nc.tensor.matmul(
                                cp[:, mc, j * BS:(j + 1) * BS],
                                W2s[:, kc, mc * P:(mc + 1) * P],
                                AT_prev[0][:, kc, :],
                                start=False,
                                stop=(j == GR - 1 and kc == 1))
                AT = atp.tile([P, 2, BS], F16, tag="AT", name="AT")
                nc.scalar.activation(
                    AT, cp[:, :, j * BS:(j + 1) * BS],
                    mybir.ActivationFunctionType.Tanh)
                AT_prev[0] = AT
                if j == 0:
                    s["yT"] = yp.tile([P, 2, TB], F32, tag="yT", name="yT")
                nc.vector.tensor_add(
                    out=s["yT"][:, :, j * BS:(j + 1) * BS],
                    in0=s["hT"][:, :, j * BS:(j + 1) * BS],
                    in1=AT)

            # ---- prologue: group 0 fully prepared before the scan ----
            for g0 in range(min(2, NG)):
                xin(g0)()
            for dc in (0, 1):
                h_mm(0, 0, dc)()
            for chunk in range(2):
                h_act(0, 0, chunk, nchunks=2)()
            for dc in (0, 1):
                h_mm(0, 1, dc)()
            if use_b2:
                for mc in (0, 1):
                    c_ones(0, mc)()
            else:
                # col-0 init for group 0 (z_0 = b2 = 0)
                for mc in (0, 1):
                    c_ones(0, mc, cols=BS)()
            for mc in (0, 1):
                c_big(0, mc, 0)()
            for chunk in range(2):
                h_act(0, 1, chunk, nchunks=2)()
            for mc in (0, 1):
                c_big(0, mc, 1)()

            # ---- scan with budgeted background drain ----
            from collections import deque
            work = deque()
            pe_cr = act_cr = dma_cr = 0.0
            for t in range(T_steps):
                g, j = divmod(t, GR)
                if j == 0:
                    work.extend(group_bg(g))
                scan_step(t)
                if g == NG - 1 and j % 4 == 3 and j < GR - 1:
                    yout_part(g, j // 4)()
                pe_cr = min(pe_cr + PE_RATE, PE_CAP)
                act_cr = min(act_cr + ACT_RATE, ACT_CAP)
                dma_cr = min(dma_cr + 1, 2)
                while work:
                    kind, cost, run = work[0]
                    if kind == "pe":
                        if pe_cr < cost:
                            break
                        pe_cr -= cost
                    elif kind == "act":
                        if act_cr < cost:
                            break
                        act_cr -= cost
                    else:
                        if dma_cr < cost:
                            break
                        dma_cr -= cost
                    work.popleft()
                    run()
            while work:
                work.popleft()[2]()
            yout_part(NG - 1, 3)()

    nc.finalize()
    return nc


_NC_CACHE = {}


def _get_nc(T_steps=T, use_b2=True, use_b1=True):
    key = (T_steps, use_b1, use_b2)
    if key not in _NC_CACHE:
        _NC_CACHE[key] = build_rnn(T_steps, use_b1=use_b1, use_b2=use_b2)
    return _NC_CACHE[key]


def kernel(x, W1, b1, W2, b2):
    Tn = x.shape[1]
    NG = Tn // GR

    x = np.asarray(x, dtype=np.float32)
    W1 = np.asarray(W1, dtype=np.float32)
    b1 = np.asarray(b1, dtype=np.float32)
    W2 = np.asarray(W2, dtype=np.float32)
    b2 = np.asarray(b2, dtype=np.float32)

    use_b1 = bool(np.any(b1))
    use_b2 = bool(np.any(b2))
    nc = _get_nc(Tn, use_b2=use_b2, use_b1=use_b1)

    # host-side pre-transposes (device layouts are partition-major)
    W1t = np.ascontiguousarray(
        W1.reshape(2, P, U).transpose(1, 0, 2)).astype(np.float16)
    W2t = np.ascontiguousarray(
        W2.reshape(2, P, U).transpose(1, 0, 2)).astype(np.float16)
    b1t = np.ascontiguousarray(b1.reshape(2, P).T)
    b2t = b2.reshape(1, U).astype(np.float16)

    in_maps = []
    for c in range(NCORES):
        xc = x[c * BS:(c + 1) * BS]  # [BS, T, D]
        # xT[g, p, dc, j*BS + b] = x[b, g*GR+j, dc*P + p]
        xt = xc.reshape(BS, NG, GR, 2, P).transpose(1, 4, 3, 2, 0)
        xt = np.ascontiguousarray(xt).astype(np.float16).reshape(NG, P, 2, TB)
        in_maps.append({
            "xT": xt, "W1t": W1t, "b1t": b1t, "W2t": W2t, "b2t": b2t,
        })
    res = bass_utils.run_bass_kernel_spmd(nc, in_maps, core_ids=list(range(NCORES)))

    out = np.empty((B, Tn, U), dtype=np.float32)
    for c in range(NCORES):
        yt = res.results[c]["yT"]  # [NG, P, 2, TB]
        # y[b, g*GR+j, mc*P + p] = yT[g, p, mc, j*BS + b]
        yc = yt.reshape(NG, P, 2, GR, BS).transpose(4, 0, 3, 2, 1)
        out[c * BS:(c + 1) * BS] = yc.reshape(BS, Tn, U)
    return out


# revision 30
# speedup vs baseline: 1.0466x; 1.0051x over previous
"""Trainium2 Bass kernel for CustomRNN:
    h = tanh(x @ W1 + b1)                         [B,T,U]
    y_t = h_t + tanh(y_{t-1} @ W2 + b2)           (scan over T, y_{-1} = 0)

Strategy (8 NeuronCores, data-parallel over batch B=256 -> 32 rows/core):
  * All device-side layouts are transposed ([u/d on partitions, (step, batch)
    on free]) and the host does the transposes with numpy before/after the
    device call, so the kernel has zero on-chip transposes and every DMA has
    a 128-wide leading dim (the sim's DMA cost is bytes / leading-dim).
  * Split state: y_t = h_t + A_t with A_t = tanh(z_t),
    z_t = b2 + h_{t-1}@W2 + A_{t-1}@W2.
    The b2 term and h@W2 terms are batched into background GEMMs that
    deposit C_t = b2 + h_{t-1}@W2 for a whole 16-step group directly into a
    PSUM bank pair; the serial scan step is then only:
        4 small matmuls (A_{t-1}@W2, start=False accumulate onto C) -> tanh
    i.e. one PE->ACT->PE round trip per step; everything else (input
    projection GEMM + tanh, C GEMMs, y = h + A adds, DMA in/out) runs in
    engine-idle windows via a cost-budgeted background work queue.
  * When b1/b2 are all-zero (they are for this problem's inputs) the bias
    ones-matmuls are dropped; PSUM zero-regions are then initialized by the
    first C matmul (start=True marks the bank pending-zero, so later
    accumulating matmuls first-touch-overwrite).
  * f16 everywhere on-chip except PSUM accumulation (f32) and y (f32).
"""

import numpy as np

import concourse.bacc as bacc
import concourse.bass as bass
import concourse.mybir as mybir
import concourse.tile as tile
from concourse import bass_utils

F32 = mybir.dt.float32
F16 = mybir.dt.float16

B, T, D, U = 256, 512, 256, 256
NCORES = 8
BS = B // NCORES   # 32 batch rows per core
GR = 16            # scan steps per group
TB = GR * BS       # free columns per group (512), col = j*BS + b
P = 128

# background work-item budget costs (ns; ~2x actual so the drain paces at
# most one big PE op into each chain idle window)
C_HMM, C_ONES, C_J0, C_BIG, C_HACT = 427, 427, 30, 402, 398
PE_RATE, PE_CAP = 460, 1400
HACT_N = 8
# h tanh column chunks per u-plane per group: sized so each insertion
# (cols*0.833+185 ns) nearly fits the chain's ~213ns ACT idle window while
# keeping total ACT busy at ~the chain latency (empirically tuned)
HACT_SPLIT = (160, 64, 64, 64, 64, 64, 32)
ACT_RATE, ACT_CAP = 420, 840


def build_rnn(T_steps=T, use_b1=True, use_b2=True):
    assert T_steps % GR == 0
    NG = T_steps // GR

    nc = bacc.Bacc("TRN2", debug=False)

    xT_d = nc.dram_tensor("xT", (NG, P, 2, TB), F16, kind="ExternalInput")
    W1_d = nc.dram_tensor("W1t", (P, 2, U), F16, kind="ExternalInput")
    b1_d = nc.dram_tensor("b1t", (P, 2), F32, kind="ExternalInput")
    W2_d = nc.dram_tensor("W2t", (P, 2, U), F16, kind="ExternalInput")
    b2_d = nc.dram_tensor("b2t", (1, U), F16, kind="ExternalInput")
    y_d = nc.dram_tensor("yT", (NG, P, 2, TB), F32, kind="ExternalOutput")
    ones_d = nc.inline_tensor(np.ones((1, TB), dtype=np.float16), "ones_row")

    with tile.TileContext(nc) as tc:
        with (
            tc.tile_pool(name="const", bufs=1) as cpool,
            tc.tile_pool(name="xT", bufs=3) as xp,
            tc.tile_pool(name="hT", bufs=3) as hp,
            tc.tile_pool(name="AT", bufs=3) as atp,
            tc.tile_pool(name="yT", bufs=2) as yp,
            tc.tile_pool(name="hps", bufs=2, space="PSUM") as hps,
            tc.tile_pool(name="cps", bufs=2, space="PSUM") as cps,
        ):
            # ---- constants (issued on Pool so the x stream owns SP) ----
            W1s = cpool.tile([P, 2, U], F16, tag="W1s")
            nc.gpsimd.dma_start(W1s, W1_d.ap())
            W2s = cpool.tile([P, 2, U], F16, tag="W2s")
            nc.gpsimd.dma_start(W2s, W2_d.ap())
            b1s = cpool.tile([P, 2], F32, tag="b1s")
            nc.gpsimd.dma_start(b1s, b1_d.ap())
            b2s = cpool.tile([1, U], F16, tag="b2s")
            nc.gpsimd.dma_start(b2s, b2_d.ap())
            ones_t = cpool.tile([1, TB], F16, tag="ones")
            nc.gpsimd.dma_start(ones_t, ones_d.ap())
            # pre-warm the ACT tanh table off the critical path
            warm = cpool.tile([1, 2], F16, tag="warm")
            nc.scalar.activation(warm, ones_t[:, 0:2],
                                 mybir.ActivationFunctionType.Tanh)

            st = [dict() for _ in range(NG)]

            # ---- background work-item constructors ----
            def xin(g):
                def run():
                    s = st[g]
                    s["xT"] = xp.tile([P, 2, TB], F16, tag="xT", name="xT")
                    nc.sync.dma_start(s["xT"], xT_d.ap()[g])
                return run

            def h_mm(g, uc, dc):
                def run():
                    s = st[g]
                    if uc == 0 and dc == 0:
                        s["hp"] = hps.tile([P, 2, TB], F32, tag="hp", name="hp")
                    nc.tensor.matmul(
                        s["hp"][:, uc, :],
                        W1s[:, dc, uc * P:(uc + 1) * P],
                        s["xT"][:, dc, :],
                        start=(dc == 0), stop=(dc == 1))
                return run

            def h_act(g, uc, half, nchunks=2, sl=None):
                HH = TB // nchunks
                def run():
                    s = st[g]
                    if uc == 0 and (half == 0 or half is None):
                        s["hT"] = hp.tile([P, 2, TB], F16, tag="hT", name="hT")
                    _sl = sl if sl is not None else slice(half * HH, (half + 1) * HH)
                    if use_b1:
                        nc.scalar.activation(
                            s["hT"][:, uc, _sl], s["hp"][:, uc, _sl],
                            mybir.ActivationFunctionType.Tanh,
                            bias=b1s[:, uc:uc + 1])
                    else:
                        nc.scalar.activation(
                            s["hT"][:, uc, _sl], s["hp"][:, uc, _sl],
                            mybir.ActivationFunctionType.Tanh)
                return run

            def c_ones(g, mc, cols=None):
                # b2 broadcast; with cols=BS used only as group-0 col-0 init
                def run():
                    s = st[g]
                    if mc == 0:
                        s["cp"] = cps.tile([P, 2, TB], F32, tag="cp", name="cp")
                    if cols is None:
                        nc.tensor.matmul(
                            s["cp"][:, mc, :], b2s[:, mc * P:(mc + 1) * P],
                            ones_t, start=True, stop=False)
                    else:
                        nc.tensor.matmul(
                            s["cp"][:, mc, 0:cols], b2s[:, mc * P:(mc + 1) * P],
                            ones_t[:, 0:cols], start=True, stop=False)
                return run

            def c_j0(g, mc, kc):
                # C col 0 of group g needs h of the last step of group g-1.
                # Without the b2 ones-matmul this is the bank's first matmul:
                # kc==0 carries start=True (pending-zero init of the bank).
                def run():
                    nc.tensor.matmul(
                        st[g]["cp"][:, mc, 0:BS],
                        W2s[:, kc, mc * P:(mc + 1) * P],
                        st[g - 1]["hT"][:, kc, TB - BS:TB],
                        start=(not use_b2 and kc == 0), stop=False)
                return run

            def c_j0_alloc(g):
                def run():
                    st[g]["cp"] = cps.tile([P, 2, TB], F32, tag="cp", name="cp")
                return run

            def c_big(g, mc, kc):
                # C cols 32:512 of group g from h cols 0:480 of group g
                def run():
                    nc.tensor.matmul(
                        st[g]["cp"][:, mc, BS:TB],
                        W2s[:, kc, mc * P:(mc + 1) * P],
                        st[g]["hT"][:, kc, 0:TB - BS],
                        start=False, stop=False)
                return run

            def yout(g):
                def run():
                    nc.gpsimd.dma_start(y_d.ap()[g], st[g]["yT"])
                return run

            def yout_part(g, q, nq=4):
                QQ = TB // nq
                def run():
                    nc.gpsimd.dma_start(
                        y_d.ap()[g][:, :, q * QQ:(q + 1) * QQ],
                        st[g]["yT"][:, :, q * QQ:(q + 1) * QQ])
                return run

            def group_bg(g):
                """Work items queued at j==0 of scan group g."""
                items = []
                if g - 1 >= 0:
                    items.append(("dma", 1, yout(g - 1)))
                if g + 2 < NG:
                    items.append(("dma", 1, xin(g + 2)))
                if g + 1 < NG:
                    for uc in (0, 1):
                        for dc in (0, 1):
                            items.append(("pe", C_HMM, h_mm(g + 1, uc, dc)))
                    for uc in (0, 1):
                        col = 0
                        for qi, cols in enumerate(HACT_SPLIT):
                            items.append(
                                ("act", int(cols * 0.833) + 185,
                                 h_act(g + 1, uc, qi if col == 0 else qi,
                                       sl=slice(col, col + cols))))
                            col += cols
                        assert col == TB
                    if use_b2:
                        for mc in (0, 1):
                            items.append(("pe", C_ONES, c_ones(g + 1, mc)))
                    else:
                        items.append(("pe", 1, c_j0_alloc(g + 1)))
                    for mc in (0, 1):
                        for kc in (0, 1):
                            items.append(("pe", C_J0, c_j0(g + 1, mc, kc)))
                    for mc in (0, 1):
                        for kc in (0, 1):
                            items.append(("pe", C_BIG, c_big(g + 1, mc, kc)))
                return items

            # ---- serial scan step ----
            AT_prev = [None]

            def scan_step(t):
                g, j = divmod(t, GR)
                s = st[g]
                cp = s["cp"]
                if t > 0:
                    for mc in (0, 1):
                        for kc in (0, 1):
                            nc.tensor.matmul(
                                cp[:, mc, j * BS:(j + 1) * BS],
                                W2s[:, kc, mc * P:(mc + 1) * P],
                                AT_prev[0][:, kc, :],
                                start=False,
                                stop=(j == GR - 1 and kc == 1))
                AT = atp.tile([P, 2, BS], F16, tag="AT", name="AT")
                nc.scalar.activation(
                    AT, cp[:, :, j * BS:(j + 1) * BS],
                    mybir.ActivationFunctionType.Tanh)
                AT_prev[0] = AT
                if j == 0:
                    s["yT"] = yp.tile([P, 2, TB], F32, tag="yT", name="yT")
                nc.vector.tensor_add(
                    out=s["yT"][:, :, j * BS:(j + 1) * BS],
                    in0=s["hT"][:, :, j * BS:(j + 1) * BS],
                    in1=AT)

            # ---- prologue: group 0 fully prepared before the scan ----
            for g0 in range(min(2, NG)):
                xin(g0)()
            for dc in (0, 1):
                h_mm(0, 0, dc)()
            for chunk in range(2):
                h_act(0, 0, chunk, nchunks=2)()
            for dc in (0, 1):
                h_mm(0, 1, dc)()
            if use_b2:
                for mc in (0, 1):
                    c_ones(0, mc)()
            else:
                # col-0 init for group 0 (z_0 = b2 = 0)
                for mc in (0, 1):
                    c_ones(0, mc, cols=BS)()
            for mc in (0, 1):
                c_big(0, mc, 0)()
            for chunk in range(2):
                h_act(0, 1, chunk, nchunks=2)()
            for mc in (0, 1):
                c_big(0, mc, 1)()

            # ---- scan with budgeted background drain ----
            from collections import deque
            work = deque()
            pe_cr = act_cr = dma_cr = 0.0
            for t in range(T_steps):
                g, j = divmod(t, GR)
                if j == 0:
                    work.extend(group_bg(g))
                scan_step(t)
                if g == NG - 1 and j % 4 == 3 and j < GR - 1:
                    yout_part(g, j // 4)()
                pe_cr = min(pe_cr + PE_RATE, PE_CAP)
                act_cr = min(act_cr + ACT_RATE, ACT_CAP)
                dma_cr = min(dma_cr + 1, 2)
                while work:
                    kind, cost, run = work[0]
                    if kind == "pe":
                        if pe_cr < cost:
                            break
                        pe_cr -= cost
                    elif kind == "act":
                        if act_cr < cost:
                            break
                        act_cr -= cost
                    else:
                        if dma_cr < cost:
                            break
                        dma_cr -= cost
                    work.popleft()
                    run()
            while work:
                work.popleft()[2]()
            yout_part(NG - 1, 3)()

    nc.finalize()
    return nc


_NC_CACHE = {}


def _get_nc(T_steps=T, use_b2=True, use_b1=None):
    if use_b1 is None:
        use_b1 = use_b2
    key = (T_steps, use_b1, use_b2)
    if key not in _NC_CACHE:
        _NC_CACHE[key] = build_rnn(T_steps, use_b1=use_b1, use_b2=use_b2)
    return _NC_CACHE[key]


def kernel(x, W1, b1, W2, b2):
    Tn = x.shape[1]
    NG = Tn // GR

    x = np.asarray(x, dtype=np.float32)
    W1 = np.asarray(W1, dtype=np.float32)
    b1 = np.asarray(b1, dtype=np.float32)
    W2 = np.asarray(W2, dtype=np.float32)
    b2 = np.asarray(b2, dtype=np.float32)

    use_b1 = bool(np.any(b1))
    use_b2 = bool(np.any(b2))
    nc = _get_nc(Tn, use_b2=use_b2, use_b1=use_b1)

    # host-side pre-transposes (device layouts are partition-major)
    W1t = np.ascontiguousarray(
        W1.reshape(2, P, U).transpose(1, 0, 2)).astype(np.float16)
    W2t = np.ascontiguousarray(
        W2.reshape(2, P, U).transpose(1, 0, 2)).astype(np.float16)
    b1t = np.ascontiguousarray(b1.reshape(2, P).T)
    b2t = b2.reshape(1, U).astype(np.float16)

    in_maps = []
    for c in range(NCORES):
        xc = x[c * BS:(c + 1) * BS]  # [BS, T, D]
        # xT[g, p, dc, j*BS + b] = x[b, g*GR+j, dc*P + p]
        xt = xc.reshape(BS, NG, GR, 2, P).transpose(1, 4, 3, 2, 0)
        xt = np.ascontiguousarray(xt).astype(np.float16).reshape(NG, P, 2, TB)
        in_maps.append({
            "xT": xt, "W1t": W1t, "b1t": b1t, "W2t": W2t, "b2t": b2t,
        })
    res = bass_utils.run_bass_kernel_spmd(nc, in_maps, core_ids=list(range(NCORES)))

    out = np.empty((B, Tn, U), dtype=np.float32)
    for c in range(NCORES):
        yt = res.results[c]["yT"]  # [NG, P, 2, TB]
        # y[b, g*GR+j, mc*P + p] = yT[g, p, mc, j*BS + b]
        yc = yt.reshape(NG, P, 2, GR, BS).transpose(4, 0, 3, 2, 1)
        out[c * BS:(c + 1) * BS] = yc.reshape(BS, Tn, U)
    return out


# revision 32
# speedup vs baseline: 1.0488x; 1.0021x over previous
"""Trainium2 Bass kernel for CustomRNN:
    h = tanh(x @ W1 + b1)                         [B,T,U]
    y_t = h_t + tanh(y_{t-1} @ W2 + b2)           (scan over T, y_{-1} = 0)

Strategy (8 NeuronCores, data-parallel over batch B=256 -> 32 rows/core):
  * All device-side layouts are transposed ([u/d on partitions, (step, batch)
    on free]) and the host does the transposes with numpy before/after the
    device call, so the kernel has zero on-chip transposes and every DMA has
    a 128-wide leading dim (the sim's DMA cost is bytes / leading-dim).
  * Split state: y_t = h_t + A_t with A_t = tanh(z_t),
    z_t = b2 + h_{t-1}@W2 + A_{t-1}@W2.
    The b2 term and h@W2 terms are batched into background GEMMs that
    deposit C_t = b2 + h_{t-1}@W2 for a whole 16-step group directly into a
    PSUM bank pair; the serial scan step is then only:
        4 small matmuls (A_{t-1}@W2, start=False accumulate onto C) -> tanh
    i.e. one PE->ACT->PE round trip per step; everything else (input
    projection GEMM + tanh, C GEMMs, y = h + A adds, DMA in/out) runs in
    engine-idle windows via a cost-budgeted background work queue.
  * When b1/b2 are all-zero (they are for this problem's inputs) the bias
    ones-matmuls are dropped; PSUM zero-regions are then initialized by the
    first C matmul (start=True marks the bank pending-zero, so later
    accumulating matmuls first-touch-overwrite).
  * f16 everywhere on-chip except PSUM accumulation (f32) and y (f32).
"""

import numpy as np

import concourse.bacc as bacc
import concourse.bass as bass
import concourse.mybir as mybir
import concourse.tile as tile
from concourse import bass_utils

F32 = mybir.dt.float32
F16 = mybir.dt.float16

B, T, D, U = 256, 512, 256, 256
NCORES = 8
BS = B // NCORES   # 32 batch rows per core
GR = 16            # scan steps per group
TB = GR * BS       # free columns per group (512), col = j*BS + b
P = 128

# background work-item budget costs (ns; ~2x actual so the drain paces at
# most one big PE op into each chain idle window)
C_HMM, C_ONES, C_J0, C_BIG, C_HACT = 427, 427, 30, 402, 398
PE_RATE, PE_CAP = 460, 1400
HACT_N = 8
# h tanh column chunks per u-plane per group: sized so each insertion
# (cols*0.833+185 ns) nearly fits the chain's ~213ns ACT idle window while
# keeping total ACT busy at ~the chain latency (empirically tuned)
HACT_SPLIT = (160, 64, 64, 64, 64, 64, 32)
ACT_RATE, ACT_CAP = 420, 840


def build_rnn(T_steps=T, use_b1=True, use_b2=True):
    assert T_steps % GR == 0
    NG = T_steps // GR

    nc = bacc.Bacc("TRN2", debug=False)

    xT_d = nc.dram_tensor("xT", (NG, P, 2, TB), F16, kind="ExternalInput")
    W1_d = nc.dram_tensor("W1t", (P, 2, U), F16, kind="ExternalInput")
    b1_d = nc.dram_tensor("b1t", (P, 2), F32, kind="ExternalInput")
    W2_d = nc.dram_tensor("W2t", (P, 2, U), F16, kind="ExternalInput")
    b2_d = nc.dram_tensor("b2t", (1, U), F16, kind="ExternalInput")
    y_d = nc.dram_tensor("yT", (NG, P, 2, TB), F32, kind="ExternalOutput")
    ones_d = nc.inline_tensor(np.ones((1, TB), dtype=np.float16), "ones_row")

    with tile.TileContext(nc) as tc:
        with (
            tc.tile_pool(name="const", bufs=1) as cpool,
            tc.tile_pool(name="xT", bufs=3) as xp,
            tc.tile_pool(name="hT", bufs=3) as hp,
            tc.tile_pool(name="AT", bufs=3) as atp,
            tc.tile_pool(name="yT", bufs=2) as yp,
            tc.tile_pool(name="hps", bufs=2, space="PSUM") as hps,
            tc.tile_pool(name="cps", bufs=2, space="PSUM") as cps,
        ):
            # ---- constants (issued on Pool so the x stream owns SP) ----
            W1s = cpool.tile([P, 2, U], F16, tag="W1s")
            nc.gpsimd.dma_start(W1s, W1_d.ap())
            W2s = cpool.tile([P, 2, U], F16, tag="W2s")
            nc.gpsimd.dma_start(W2s, W2_d.ap())
            b1s = cpool.tile([P, 2], F32, tag="b1s")
            nc.gpsimd.dma_start(b1s, b1_d.ap())
            b2s = cpool.tile([1, U], F16, tag="b2s")
            nc.gpsimd.dma_start(b2s, b2_d.ap())
            ones_t = cpool.tile([1, TB], F16, tag="ones")
            nc.gpsimd.dma_start(ones_t, ones_d.ap())
            # pre-warm the ACT tanh table off the critical path
            warm = cpool.tile([1, 2], F16, tag="warm")
            nc.scalar.activation(warm, ones_t[:, 0:2],
                                 mybir.ActivationFunctionType.Tanh)

            st = [dict() for _ in range(NG)]

            # ---- background work-item constructors ----
            def xin(g):
                def run():
                    s = st[g]
                    s["xT"] = xp.tile([P, 2, TB], F16, tag="xT", name="xT")
                    nc.sync.dma_start(s["xT"], xT_d.ap()[g])
                return run

            def h_mm(g, uc, dc):
                def run():
                    s = st[g]
                    if uc == 0 and dc == 0:
                        s["hp"] = hps.tile([P, 2, TB], F32, tag="hp", name="hp")
                    nc.tensor.matmul(
                        s["hp"][:, uc, :],
                        W1s[:, dc, uc * P:(uc + 1) * P],
                        s["xT"][:, dc, :],
                        start=(dc == 0), stop=(dc == 1))
                return run

            def h_act(g, uc, half, nchunks=2, sl=None):
                HH = TB // nchunks
                def run():
                    s = st[g]
                    if uc == 0 and (half == 0 or half is None):
                        s["hT"] = hp.tile([P, 2, TB], F16, tag="hT", name="hT")
                    _sl = sl if sl is not None else slice(half * HH, (half + 1) * HH)
                    if use_b1:
                        nc.scalar.activation(
                            s["hT"][:, uc, _sl], s["hp"][:, uc, _sl],
                            mybir.ActivationFunctionType.Tanh,
                            bias=b1s[:, uc:uc + 1])
                    else:
                        nc.scalar.activation(
                            s["hT"][:, uc, _sl], s["hp"][:, uc, _sl],
                            mybir.ActivationFunctionType.Tanh)
                return run

            def c_ones(g, mc, cols=None):
                # b2 broadcast; with cols=BS used only as group-0 col-0 init
                def run():
                    s = st[g]
                    if mc == 0:
                        s["cp"] = cps.tile([P, 2, TB], F32, tag="cp", name="cp")
                    if cols is None:
                        nc.tensor.matmul(
                            s["cp"][:, mc, :], b2s[:, mc * P:(mc + 1) * P],
                            ones_t, start=True, stop=False)
                    else:
                        nc.tensor.matmul(
                            s["cp"][:, mc, 0:cols], b2s[:, mc * P:(mc + 1) * P],
                            ones_t[:, 0:cols], start=True, stop=False)
                return run

            def c_j0(g, mc, kc):
                # C col 0 of group g needs h of the last step of group g-1.
                # Without the b2 ones-matmul this is the bank's first matmul:
                # kc==0 carries start=True (pending-zero init of the bank).
                def run():
                    nc.tensor.matmul(
                        st[g]["cp"][:, mc, 0:BS],
                        W2s[:, kc, mc * P:(mc + 1) * P],
                        st[g - 1]["hT"][:, kc, TB - BS:TB],
                        start=(not use_b2 and kc == 0), stop=False)
                return run

            def c_j0_alloc(g):
                def run():
                    st[g]["cp"] = cps.tile([P, 2, TB], F32, tag="cp", name="cp")
                return run

            def c_big(g, mc, kc):
                # C cols 32:512 of group g from h cols 0:480 of group g
                def run():
                    nc.tensor.matmul(
                        st[g]["cp"][:, mc, BS:TB],
                        W2s[:, kc, mc * P:(mc + 1) * P],
                        st[g]["hT"][:, kc, 0:TB - BS],
                        start=False, stop=False)
                return run

            def yout(g):
                def run():
                    nc.gpsimd.dma_start(y_d.ap()[g], st[g]["yT"])
                return run

            def yout_part(g, q, nq=4):
                QQ = TB // nq
                def run():
                    nc.gpsimd.dma_start(
                        y_d.ap()[g][:, :, q * QQ:(q + 1) * QQ],
                        st[g]["yT"][:, :, q * QQ:(q + 1) * QQ])
                return run

            def group_bg(g):
                """Work items queued at j==0 of scan group g."""
                items = []
                if g - 1 >= 0:
                    items.append(("dma", 1, yout(g - 1)))
                if g + 2 < NG:
                    items.append(("dma", 1, xin(g + 2)))
                if g + 1 < NG:
                    for uc in (0, 1):
                        for dc in (0, 1):
                            items.append(("pe", C_HMM, h_mm(g + 1, uc, dc)))
                    for uc in (0, 1):
                        col = 0
                        for qi, cols in enumerate(HACT_SPLIT):
                            items.append(
                                ("act", int(cols * 0.833) + 185,
                                 h_act(g + 1, uc, qi if col == 0 else qi,
                                       sl=slice(col, col + cols))))
                            col += cols
                        assert col == TB
                    if use_b2:
                        for mc in (0, 1):
                            items.append(("pe", C_ONES, c_ones(g + 1, mc)))
                    else:
                        items.append(("pe", 1, c_j0_alloc(g + 1)))
                    for mc in (0, 1):
                        for kc in (0, 1):
                            items.append(("pe", C_J0, c_j0(g + 1, mc, kc)))
                    for mc in (0, 1):
                        for kc in (0, 1):
                            items.append(("pe", C_BIG, c_big(g + 1, mc, kc)))
                return items

            # ---- serial scan step ----
            AT_prev = [None]

            def scan_step(t):
                g, j = divmod(t, GR)
                s = st[g]
                cp = s["cp"]
                if t > 0:
                    for mc in (0, 1):
                        for kc in (0, 1):
                            nc.tensor.matmul(
                                cp[:, mc, j * BS:(j + 1) * BS],
                                W2s[:, kc, mc * P:(mc + 1) * P],
                                AT_prev[0][:, kc, :],
                                start=False,
                                stop=(j == GR - 1 and kc == 1))
                AT = atp.tile([P, 2, BS], F16, tag="AT", name="AT")
                nc.scalar.activation(
                    AT, cp[:, :, j * BS:(j + 1) * BS],
                    mybir.ActivationFunctionType.Tanh)
                AT_prev[0] = AT
                if j == 0:
                    s["yT"] = yp.tile([P, 2, TB], F32, tag="yT", name="yT")
                nc.vector.tensor_add(
                    out=s["yT"][:, :, j * BS:(j + 1) * BS],
                    in0=s["hT"][:, :, j * BS:(j + 1) * BS],
                    in1=AT)

            # ---- prologue: group 0 fully prepared before the scan ----
            for g0 in range(min(2, NG)):
                xin(g0)()
            for dc in (0, 1):
                h_mm(0, 0, dc)()
            for chunk in range(2):
                h_act(0, 0, chunk, nchunks=2)()
            for dc in (0, 1):
                h_mm(0, 1, dc)()
            if use_b2:
                for mc in (0, 1):
                    c_ones(0, mc)()
            else:
                # col-0 init for group 0 (z_0 = b2 = 0)
                for mc in (0, 1):
                    c_ones(0, mc, cols=BS)()
            for chunk in range(2):
                h_act(0, 1, chunk, nchunks=2)()
            cbig0_tail = [c_big(0, mc, kc) for kc in (0, 1) for mc in (0, 1)]

            # ---- scan with budgeted background drain ----
            from collections import deque
            work = deque()
            pe_cr = act_cr = dma_cr = 0.0
            for t in range(T_steps):
                g, j = divmod(t, GR)
                if j == 0:
                    work.extend(group_bg(g))
                scan_step(t)
                if t == 0:
                    for fn in cbig0_tail:
                        fn()
                if g == NG - 1 and j % 4 == 3 and j < GR - 1:
                    yout_part(g, j // 4)()
                pe_cr = min(pe_cr + PE_RATE, PE_CAP)
                act_cr = min(act_cr + ACT_RATE, ACT_CAP)
                dma_cr = min(dma_cr + 1, 2)
                while work:
                    kind, cost, run = work[0]
                    if kind == "pe":
                        if pe_cr < cost:
                            break
                        pe_cr -= cost
                    elif kind == "act":
                        if act_cr < cost:
                            break
                        act_cr -= cost
                    else:
                        if dma_cr < cost:
                            break
                        dma_cr -= cost
                    work.popleft()
                    run()
            while work:
                work.popleft()[2]()
            yout_part(NG - 1, 3)()

    nc.finalize()
    return nc


_NC_CACHE = {}


def _get_nc(T_steps=T, use_b2=True, use_b1=None):
    if use_b1 is None:
        use_b1 = use_b2
    key = (T_steps, use_b1, use_b2)
    if key not in _NC_CACHE:
        _NC_CACHE[key] = build_rnn(T_steps, use_b1=use_b1, use_b2=use_b2)
    return _NC_CACHE[key]


def kernel(x, W1, b1, W2, b2):
    Tn = x.shape[1]
    NG = Tn // GR

    x = np.asarray(x, dtype=np.float32)
    W1 = np.asarray(W1, dtype=np.float32)
    b1 = np.asarray(b1, dtype=np.float32)
    W2 = np.asarray(W2, dtype=np.float32)
    b2 = np.asarray(b2, dtype=np.float32)

    use_b1 = bool(np.any(b1))
    use_b2 = bool(np.any(b2))
    nc = _get_nc(Tn, use_b2=use_b2, use_b1=use_b1)

    # host-side pre-transposes (device layouts are partition-major)
    W1t = np.ascontiguousarray(
        W1.reshape(2, P, U).transpose(1, 0, 2)).astype(np.float16)
    W2t = np.ascontiguousarray(
        W2.reshape(2, P, U).transpose(1, 0, 2)).astype(np.float16)
    b1t = np.ascontiguousarray(b1.reshape(2, P).T)
    b2t = b2.reshape(1, U).astype(np.float16)

    in_maps = []
    for c in range(NCORES):
        xc = x[c * BS:(c + 1) * BS]  # [BS, T, D]
        # xT[g, p, dc, j*BS + b] = x[b, g*GR+j, dc*P + p]
        xt = xc.reshape(BS, NG, GR, 2, P).transpose(1, 4, 3, 2, 0)
        xt = np.ascontiguousarray(xt).astype(np.float16).reshape(NG, P, 2, TB)
        in_maps.append({
            "xT": xt, "W1t": W1t, "b1t": b1t, "W2t": W2t, "b2t": b2t,
        })
    res = bass_utils.run_bass_kernel_spmd(nc, in_maps, core_ids=list(range(NCORES)))

    out = np.empty((B, Tn, U), dtype=np.float32)
    for c in range(NCORES):
        yt = res.results[c]["yT"]  # [NG, P, 2, TB]
        # y[b, g*GR+j, mc*P + p] = yT[g, p, mc, j*BS + b]
        yc = yt.reshape(NG, P, 2, GR, BS).transpose(4, 0, 3, 2, 1)
        out[c * BS:(c + 1) * BS] = yc.reshape(BS, Tn, U)
    return out


# revision 33
# speedup vs baseline: 1.0489x; 1.0001x over previous
"""Trainium2 Bass kernel for CustomRNN:
    h = tanh(x @ W1 + b1)                         [B,T,U]
    y_t = h_t + tanh(y_{t-1} @ W2 + b2)           (scan over T, y_{-1} = 0)

Strategy (8 NeuronCores, data-parallel over batch B=256 -> 32 rows/core):
  * All device-side layouts are transposed ([u/d on partitions, (step, batch)
    on free]) and the host does the transposes with numpy before/after the
    device call, so the kernel has zero on-chip transposes and every DMA has
    a 128-wide leading dim (the sim's DMA cost is bytes / leading-dim).
  * Split state: y_t = h_t + A_t with A_t = tanh(z_t),
    z_t = b2 + h_{t-1}@W2 + A_{t-1}@W2.
    The b2 term and h@W2 terms are batched into background GEMMs that
    deposit C_t = b2 + h_{t-1}@W2 for a whole 16-step group directly into a
    PSUM bank pair; the serial scan step is then only:
        4 small matmuls (A_{t-1}@W2, start=False accumulate onto C) -> tanh
    i.e. one PE->ACT->PE round trip per step; everything else (input
    projection GEMM + tanh, C GEMMs, y = h + A adds, DMA in/out) runs in
    engine-idle windows via a cost-budgeted background work queue.
  * When b1/b2 are all-zero (they are for this problem's inputs) the bias
    ones-matmuls are dropped; PSUM zero-regions are then initialized by the
    first C matmul (start=True marks the bank pending-zero, so later
    accumulating matmuls first-touch-overwrite).
  * f16 everywhere on-chip except PSUM accumulation (f32) and y (f32).
"""

import numpy as np

import concourse.bacc as bacc
import concourse.bass as bass
import concourse.mybir as mybir
import concourse.tile as tile
from concourse import bass_utils

F32 = mybir.dt.float32
F16 = mybir.dt.float16

B, T, D, U = 256, 512, 256, 256
NCORES = 8
BS = B // NCORES   # 32 batch rows per core
GR = 16            # scan steps per group
TB = GR * BS       # free columns per group (512), col = j*BS + b
P = 128

# background work-item budget costs (ns; ~2x actual so the drain paces at
# most one big PE op into each chain idle window)
C_HMM, C_ONES, C_J0, C_BIG, C_HACT = 427, 427, 30, 402, 398
PE_RATE, PE_CAP = 460, 1400
HACT_N = 8
# h tanh column chunks per u-plane per group: sized so each insertion
# (cols*0.833+185 ns) nearly fits the chain's ~213ns ACT idle window while
# keeping total ACT busy at ~the chain latency (empirically tuned)
HACT_SPLIT = (64, 64, 64, 160, 64, 64, 32)
ACT_RATE, ACT_CAP = 420, 840


def build_rnn(T_steps=T, use_b1=True, use_b2=True):
    assert T_steps % GR == 0
    NG = T_steps // GR

    nc = bacc.Bacc("TRN2", debug=False)

    xT_d = nc.dram_tensor("xT", (NG, P, 2, TB), F16, kind="ExternalInput")
    W1_d = nc.dram_tensor("W1t", (P, 2, U), F16, kind="ExternalInput")
    b1_d = nc.dram_tensor("b1t", (P, 2), F32, kind="ExternalInput")
    W2_d = nc.dram_tensor("W2t", (P, 2, U), F16, kind="ExternalInput")
    b2_d = nc.dram_tensor("b2t", (1, U), F16, kind="ExternalInput")
    y_d = nc.dram_tensor("yT", (NG, P, 2, TB), F32, kind="ExternalOutput")
    ones_d = nc.inline_tensor(np.ones((1, TB), dtype=np.float16), "ones_row")

    with tile.TileContext(nc) as tc:
        with (
            tc.tile_pool(name="const", bufs=1) as cpool,
            tc.tile_pool(name="xT", bufs=3) as xp,
            tc.tile_pool(name="hT", bufs=3) as hp,
            tc.tile_pool(name="AT", bufs=3) as atp,
            tc.tile_pool(name="yT", bufs=2) as yp,
            tc.tile_pool(name="hps", bufs=2, space="PSUM") as hps,
            tc.tile_pool(name="cps", bufs=2, space="PSUM") as cps,
        ):
            # ---- constants (issued on Pool so the x stream owns SP) ----
            W1s = cpool.tile([P, 2, U], F16, tag="W1s")
            nc.gpsimd.dma_start(W1s, W1_d.ap())
            W2s = cpool.tile([P, 2, U], F16, tag="W2s")
            nc.gpsimd.dma_start(W2s, W2_d.ap())
            b1s = cpool.tile([P, 2], F32, tag="b1s")
            nc.gpsimd.dma_start(b1s, b1_d.ap())
            b2s = cpool.tile([1, U], F16, tag="b2s")
            nc.gpsimd.dma_start(b2s, b2_d.ap())
            ones_t = cpool.tile([1, TB], F16, tag="ones")
            nc.gpsimd.dma_start(ones_t, ones_d.ap())
            # pre-warm the ACT tanh table off the critical path
            warm = cpool.tile([1, 2], F16, tag="warm")
            nc.scalar.activation(warm, ones_t[:, 0:2],
                                 mybir.ActivationFunctionType.Tanh)

            st = [dict() for _ in range(NG)]

            # ---- background work-item constructors ----
            def xin(g):
                def run():
                    s = st[g]
                    s["xT"] = xp.tile([P, 2, TB], F16, tag="xT", name="xT")
                    nc.sync.dma_start(s["xT"], xT_d.ap()[g])
                return run

            def h_mm(g, uc, dc):
                def run():
                    s = st[g]
                    if uc == 0 and dc == 0:
                        s["hp"] = hps.tile([P, 2, TB], F32, tag="hp", name="hp")
                    nc.tensor.matmul(
                        s["hp"][:, uc, :],
                        W1s[:, dc, uc * P:(uc + 1) * P],
                        s["xT"][:, dc, :],
                        start=(dc == 0), stop=(dc == 1))
                return run

            def h_act(g, uc, half, nchunks=2, sl=None):
                HH = TB // nchunks
                def run():
                    s = st[g]
                    if uc == 0 and (half == 0 or half is None):
                        s["hT"] = hp.tile([P, 2, TB], F16, tag="hT", name="hT")
                    _sl = sl if sl is not None else slice(half * HH, (half + 1) * HH)
                    if use_b1:
                        nc.scalar.activation(
                            s["hT"][:, uc, _sl], s["hp"][:, uc, _sl],
                            mybir.ActivationFunctionType.Tanh,
                            bias=b1s[:, uc:uc + 1])
                    else:
                        nc.scalar.activation(
                            s["hT"][:, uc, _sl], s["hp"][:, uc, _sl],
                            mybir.ActivationFunctionType.Tanh)
                return run

            def c_ones(g, mc, cols=None):
                # b2 broadcast; with cols=BS used only as group-0 col-0 init
                def run():
                    s = st[g]
                    if mc == 0:
                        s["cp"] = cps.tile([P, 2, TB], F32, tag="cp", name="cp")
                    if cols is None:
                        nc.tensor.matmul(
                            s["cp"][:, mc, :], b2s[:, mc * P:(mc + 1) * P],
                            ones_t, start=True, stop=False)
                    else:
                        nc.tensor.matmul(
                            s["cp"][:, mc, 0:cols], b2s[:, mc * P:(mc + 1) * P],
                            ones_t[:, 0:cols], start=True, stop=False)
                return run

            def c_j0(g, mc, kc):
                # C col 0 of group g needs h of the last step of group g-1.
                # Without the b2 ones-matmul this is the bank's first matmul:
                # kc==0 carries start=True (pending-zero init of the bank).
                def run():
                    nc.tensor.matmul(
                        st[g]["cp"][:, mc, 0:BS],
                        W2s[:, kc, mc * P:(mc + 1) * P],
                        st[g - 1]["hT"][:, kc, TB - BS:TB],
                        start=(not use_b2 and kc == 0), stop=False)
                return run

            def c_j0_alloc(g):
                def run():
                    st[g]["cp"] = cps.tile([P, 2, TB], F32, tag="cp", name="cp")
                return run

            def c_big(g, mc, kc):
                # C cols 32:512 of group g from h cols 0:480 of group g
                def run():
                    nc.tensor.matmul(
                        st[g]["cp"][:, mc, BS:TB],
                        W2s[:, kc, mc * P:(mc + 1) * P],
                        st[g]["hT"][:, kc, 0:TB - BS],
                        start=False, stop=False)
                return run

            def yout(g):
                def run():
                    nc.gpsimd.dma_start(y_d.ap()[g], st[g]["yT"])
                return run

            def yout_part(g, q, nq=4):
                QQ = TB // nq
                def run():
                    nc.gpsimd.dma_start(
                        y_d.ap()[g][:, :, q * QQ:(q + 1) * QQ],
                        st[g]["yT"][:, :, q * QQ:(q + 1) * QQ])
                return run

            def group_bg(g):
                """Work items queued at j==0 of scan group g."""
                items = []
                if g - 1 >= 0:
                    items.append(("dma", 1, yout(g - 1)))
                if g + 2 < NG:
                    items.append(("dma", 1, xin(g + 2)))
                if g + 1 < NG:
                    for uc in (0, 1):
                        for dc in (0, 1):
                            items.append(("pe", C_HMM, h_mm(g + 1, uc, dc)))
                    for uc in (0, 1):
                        col = 0
                        for qi, cols in enumerate(HACT_SPLIT):
                            items.append(
                                ("act", int(cols * 0.833) + 185,
                                 h_act(g + 1, uc, qi if col == 0 else qi,
                                       sl=slice(col, col + cols))))
                            col += cols
                        assert col == TB
                    if use_b2:
                        for mc in (0, 1):
                            items.append(("pe", C_ONES, c_ones(g + 1, mc)))
                    else:
                        items.append(("pe", 1, c_j0_alloc(g + 1)))
                    for mc in (0, 1):
                        for kc in (0, 1):
                            items.append(("pe", C_J0, c_j0(g + 1, mc, kc)))
                    for mc in (0, 1):
                        for kc in (0, 1):
                            items.append(("pe", C_BIG, c_big(g + 1, mc, kc)))
                return items

            # ---- serial scan step ----
            AT_prev = [None]

            def scan_step(t):
                g, j = divmod(t, GR)
                s = st[g]
                cp = s["cp"]
                if t > 0:
                    for mc in (0, 1):
                        for kc in (0, 1):
                            nc.tensor.matmul(
                                cp[:, mc, j * BS:(j + 1) * BS],
                                W2s[:, kc, mc * P:(mc + 1) * P],
                                AT_prev[0][:, kc, :],
                                start=False,
                                stop=(j == GR - 1 and kc == 1))
                AT = atp.tile([P, 2, BS], F16, tag="AT", name="AT")
                nc.scalar.activation(
                    AT, cp[:, :, j * BS:(j + 1) * BS],
                    mybir.ActivationFunctionType.Tanh)
                AT_prev[0] = AT
                if j == 0:
                    s["yT"] = yp.tile([P, 2, TB], F32, tag="yT", name="yT")
                nc.vector.tensor_add(
                    out=s["yT"][:, :, j * BS:(j + 1) * BS],
                    in0=s["hT"][:, :, j * BS:(j + 1) * BS],
                    in1=AT)

            # ---- prologue: group 0 fully prepared before the scan ----
            for g0 in range(min(2, NG)):
                xin(g0)()
            for dc in (0, 1):
                h_mm(0, 0, dc)()
            for chunk in range(2):
                h_act(0, 0, chunk, nchunks=2)()
            for dc in (0, 1):
                h_mm(0, 1, dc)()
            if use_b2:
                for mc in (0, 1):
                    c_ones(0, mc)()
            else:
                # col-0 init for group 0 (z_0 = b2 = 0)
                for mc in (0, 1):
                    c_ones(0, mc, cols=BS)()
            for chunk in range(2):
                h_act(0, 1, chunk, nchunks=2)()
            cbig0_tail = [c_big(0, mc, kc) for kc in (0, 1) for mc in (0, 1)]

            # ---- scan with budgeted background drain ----
            from collections import deque
            work = deque()
            pe_cr = act_cr = dma_cr = 0.0
            for t in range(T_steps):
                g, j = divmod(t, GR)
                if j == 0:
                    work.extend(group_bg(g))
                scan_step(t)
                if t == 0:
                    for fn in cbig0_tail:
                        fn()
                if g == NG - 1 and j % 4 == 3 and j < GR - 1:
                    yout_part(g, j // 4)()
                pe_cr = min(pe_cr + PE_RATE, PE_CAP)
                act_cr = min(act_cr + ACT_RATE, ACT_CAP)
                dma_cr = min(dma_cr + 1, 2)
                while work:
                    kind, cost, run = work[0]
                    if kind == "pe":
                        if pe_cr < cost:
                            break
                        pe_cr -= cost
                    elif kind == "act":
                        if act_cr < cost:
                            break
                        act_cr -= cost
                    else:
                        if dma_cr < cost:
                            break
                        dma_cr -= cost
                    work.popleft()
                    run()
            while work:
                work.popleft()[2]()
            yout_part(NG - 1, 3)()

    nc.finalize()
    return nc


_NC_CACHE = {}


def _get_nc(T_steps=T, use_b2=True, use_b1=None):
    if use_b1 is None:
        use_b1 = use_b2
    key = (T_steps, use_b1, use_b2)
    if key not in _NC_CACHE:
        _NC_CACHE[key] = build_rnn(T_steps, use_b1=use_b1, use_b2=use_b2)
    return _NC_CACHE[key]


def kernel(x, W1, b1, W2, b2):
    Tn = x.shape[1]
    NG = Tn // GR

    x = np.asarray(x, dtype=np.float32)
    W1 = np.asarray(W1, dtype=np.float32)
    b1 = np.asarray(b1, dtype=np.float32)
    W2 = np.asarray(W2, dtype=np.float32)
    b2 = np.asarray(b2, dtype=np.float32)

    use_b1 = bool(np.any(b1))
    use_b2 = bool(np.any(b2))
    nc = _get_nc(Tn, use_b2=use_b2, use_b1=use_b1)

    # host-side pre-transposes (device layouts are partition-major)
    W1t = np.ascontiguousarray(
        W1.reshape(2, P, U).transpose(1, 0, 2)).astype(np.float16)
    W2t = np.ascontiguousarray(
        W2.reshape(2, P, U).transpose(1, 0, 2)).astype(np.float16)
    b1t = np.ascontiguousarray(b1.reshape(2, P).T)
    b2t = b2.reshape(1, U).astype(np.float16)

    in_maps = []
    for c in range(NCORES):
        xc = x[c * BS:(c + 1) * BS]  # [BS, T, D]
        # xT[g, p, dc, j*BS + b] = x[b, g*GR+j, dc*P + p]
        xt = xc.reshape(BS, NG, GR, 2, P).transpose(1, 4, 3, 2, 0)
        xt = np.ascontiguousarray(xt).astype(np.float16).reshape(NG, P, 2, TB)
        in_maps.append({
            "xT": xt, "W1t": W1t, "b1t": b1t, "W2t": W2t, "b2t": b2t,
        })
    res = bass_utils.run_bass_kernel_spmd(nc, in_maps, core_ids=list(range(NCORES)))

    out = np.empty((B, Tn, U), dtype=np.float32)
    for c in range(NCORES):
        yt = res.results[c]["yT"]  # [NG, P, 2, TB]
        # y[b, g*GR+j, mc*P + p] = yT[g, p, mc, j*BS + b]
        yc = yt.reshape(NG, P, 2, GR, BS).transpose(4, 0, 3, 2, 1)
        out[c * BS:(c + 1) * BS] = yc.reshape(BS, Tn, U)
    return out
